# revision 15
# baseline (speedup 1.0000x reference)
"""Trainium2 Bass kernel for nn_GAT_66821101191795 (2-layer GAT, 8 NeuronCores).

Strategy (graph/data parallel, dst-sharded):
- Host: add self loops, sort edges by dst, shard dst nodes into 8 blocks of
  2500, pack each destination node's edges into contiguous slots of 128-slot
  chunks (<=16 dst nodes per chunk for layer 1, <=32 for layer 2). Per-edge
  source features are gathered host-side ("all-to-all the gathered source
  features") into per-slot fp16 tiles; attention a-values are likewise
  expanded per slot. Weight reparameterisations: vsrc/vdst = att @ W folds so
  attention logits come from emb directly; W2.T @ att2 folds the layer-2
  attention projections.
- Launch A (device): node/col encoders -> emb1^T shard + a1^T shard per core.
- Launch B (device): layer-1 attention (leaky+exp+softmax via masked
  numerators and a ones-matmul for denominators), aggregation as one
  128x128x128 fp16 matmul per chunk, PE transpose, W1 apply + bias + relu,
  xp2 = emb2 @ W2.T and a2 = emb2 @ w2v contractions.
- Launch C (device): layer-2 attention + aggregation (+b2, relu) + final
  linear -> logits^T slots. Host unpacks slots -> logits [10000, 128].
"""

import sys

for _p in ("/opt/trn_rl_repo", "/root/.axon_site"):
    if _p not in sys.path:
        sys.path.insert(0, _p)

import numpy as np

import concourse.bacc as bacc
import concourse.bass as bass
import concourse.tile as tile
from concourse import mybir
from concourse.bass_utils import run_bass_kernel_spmd

F32 = mybir.dt.float32
F16 = mybir.dt.float16

N_CONS = 10000
N_COLS = 10000
N = N_CONS + N_COLS
N_CORES = 8
SHARD = N // N_CORES
NEG = 0.2
GB = 16            # chunks per compute batch
WB = 8             # chunks per W1 batch (launch B)
ENC_COLS = 2560    # padded shard width for launch A (5 x 512)

_programs = {}


# ----------------------------------------------------------------------------
# host-side edge preprocessing
# ----------------------------------------------------------------------------

def _pack_edges(src, dst, lo, hi, max_nodes):
    """Pack edges with dst in [lo, hi) into 128-slot chunks.

    Each dst node's edges occupy contiguous slots within a single chunk; at
    most max_nodes nodes per chunk.
    """
    sel = (dst >= lo) & (dst < hi)
    s = src[sel]
    d = dst[sel]
    order = np.argsort(d, kind="stable")
    s = s[order]
    d = d[order]
    nodes, counts = np.unique(d, return_counts=True)
    assert counts.max() <= 128, f"degree {counts.max()} > 128 unsupported"
    offs = np.concatenate([[0], np.cumsum(counts)])

    chunk_src = []
    chunk_nodes = []
    cur_src, cur_nodes, cur_slots = [], [], 0
    for i in range(len(nodes)):
        k = int(counts[i])
        if cur_slots + k > 128 or len(cur_nodes) >= max_nodes:
            chunk_src.append(cur_src)
            chunk_nodes.append(cur_nodes)
            cur_src, cur_nodes, cur_slots = [], [], 0
        cur_src.append(s[offs[i]:offs[i + 1]])
        cur_nodes.append((int(nodes[i]), k))
        cur_slots += k
    if cur_nodes:
        chunk_src.append(cur_src)
        chunk_nodes.append(cur_nodes)

    nc_ = len(chunk_nodes)
    src_idx = np.zeros(128 * nc_, np.int64)
    dst_idx = np.zeros(128 * nc_, np.int64)
    node_col = np.full(128 * nc_, -1, np.int32)
    node_map = np.full(nc_ * max_nodes, -1, np.int32)
    for c in range(nc_):
        slot = 0
        for j, (nd, k) in enumerate(chunk_nodes[c]):
            sl = slice(128 * c + slot, 128 * c + slot + k)
            src_idx[sl] = chunk_src[c][j]
            dst_idx[sl] = nd
            node_col[sl] = j
            node_map[c * max_nodes + j] = nd
            slot += k
    return dict(n_chunks=nc_, src_idx=src_idx, dst_idx=dst_idx,
                node_col=node_col, node_map=node_map, max_nodes=max_nodes)


def _pad_chunks(pk, n_chunks_to):
    nc_, mx = pk["n_chunks"], pk["max_nodes"]
    pad = n_chunks_to - nc_
    assert pad >= 0
    if pad:
        z = np.zeros(128 * pad, np.int64)
        pk["src_idx"] = np.concatenate([pk["src_idx"], z])
        pk["dst_idx"] = np.concatenate([pk["dst_idx"], z])
        pk["node_col"] = np.concatenate(
            [pk["node_col"], np.full(128 * pad, -1, np.int32)])
        pk["node_map"] = np.concatenate(
            [pk["node_map"], np.full(mx * pad, -1, np.int32)])
    pk["n_chunks"] = n_chunks_to
    return pk


def _expand_slots(pk, table, dtype):
    """Per-slot rows table[src_idx] laid out [128, nc * width]."""
    nc_ = pk["n_chunks"]
    w = table.shape[1]
    t = table[pk["src_idx"]].reshape(nc_, 128, w).transpose(1, 0, 2)
    return np.ascontiguousarray(t.reshape(128, nc_ * w), dtype)


def _mask01(pk, dtype):
    """indicator mask [128, nc*max_nodes]: 1.0 at the slot's node col."""
    nc_, mx = pk["n_chunks"], pk["max_nodes"]
    ncol = pk["node_col"].reshape(nc_, 128)
    cols = np.arange(mx)
    m = (ncol[:, :, None] == cols[None, None, :]).astype(np.float32)
    out = m.transpose(1, 0, 2).reshape(128, nc_ * mx)
    return np.ascontiguousarray(out, dtype)


def _leaky_np(x):
    return np.where(x > 0, x, NEG * x).astype(np.float32)


# ----------------------------------------------------------------------------
# launch A: encoders
# ----------------------------------------------------------------------------

def _build_launch_a():
    nc = bacc.Bacc("TRN2", target_bir_lowering=False, debug=False)
    encT = nc.dram_tensor("encT", [16, ENC_COLS], F32, kind="ExternalInput").ap()
    encWT = nc.dram_tensor("encWT", [16, 128], F32, kind="ExternalInput").ap()
    encb = nc.dram_tensor("encb", [128, 1], F32, kind="ExternalInput").ap()
    vsV = nc.dram_tensor("vsV", [128, 16], F32, kind="ExternalInput").ap()
    embo = nc.dram_tensor("embo", [128, ENC_COLS], F32, kind="ExternalOutput").ap()
    a1o = nc.dram_tensor("a1o", [16, ENC_COLS], F32, kind="ExternalOutput").ap()

    with tile.TileContext(nc) as tc:
        with (
            tc.tile_pool(name="singles", bufs=1) as singles,
            tc.tile_pool(name="ps1", bufs=2, space="PSUM") as ps1,
            tc.tile_pool(name="ps2", bufs=2, space="PSUM") as ps2,
        ):
            encT_sb = singles.tile([16, ENC_COLS], F32)
            nc.sync.dma_start(out=encT_sb, in_=encT)
            encWT_sb = singles.tile([16, 128], F32)
            nc.sync.dma_start(out=encWT_sb, in_=encWT)
            encb_sb = singles.tile([128, 1], F32)
            nc.sync.dma_start(out=encb_sb, in_=encb)
            vsV_sb = singles.tile([128, 16], F32)
            nc.sync.dma_start(out=vsV_sb, in_=vsV)
            emb_sb = singles.tile([128, ENC_COLS], F32)
            a1_sb = singles.tile([16, ENC_COLS], F32)

            nw = ENC_COLS // 512
            for w in range(nw):
                sl = slice(512 * w, 512 * (w + 1))
                p1 = ps1.tile([128, 512], F32)
                nc.tensor.matmul(out=p1, lhsT=encWT_sb, rhs=encT_sb[:, sl],
                                 start=True, stop=True)
                nc.scalar.activation(emb_sb[:, sl], p1,
                                     mybir.ActivationFunctionType.Relu,
                                     bias=encb_sb[:, 0:1])
            for w in range(nw):
                sl = slice(512 * w, 512 * (w + 1))
                p2 = ps2.tile([16, 512], F32)
                nc.tensor.matmul(out=p2, lhsT=vsV_sb, rhs=emb_sb[:, sl],
                                 start=True, stop=True)
                nc.vector.tensor_copy(a1_sb[:, sl], p2)
            nc.sync.dma_start(out=embo, in_=emb_sb)
            nc.sync.dma_start(out=a1o, in_=a1_sb)
    nc.compile()
    return nc


# ----------------------------------------------------------------------------
# launch B: GAT layer 1 (+ W1, relu, xp2, a2)
# ----------------------------------------------------------------------------

def _build_launch_b(nchunks):
    assert nchunks % GB == 0
    nsn = nchunks * 16
    nwb = nchunks // WB

    nc = bacc.Bacc("TRN2", target_bir_lowering=False, debug=False)
    t_gx = nc.dram_tensor("gx", [128, nchunks * 132], F16,
                          kind="ExternalInput").ap()
    t_ap = nc.dram_tensor("apn", [128, nchunks * 16], F32,
                          kind="ExternalInput").ap()
    t_mask = nc.dram_tensor("mask01", [128, nchunks * 16], F16,
                            kind="ExternalInput").ap()
    t_mxr = nc.dram_tensor("mxr", [128, 8], F32, kind="ExternalInput").ap()
    t_w1t = nc.dram_tensor("w1t", [128, 8, 128], F16, kind="ExternalInput").ap()
    t_w2tv = nc.dram_tensor("w2tv", [128, 8, 132], F16,
                            kind="ExternalInput").ap()
    t_b1c = nc.dram_tensor("b1c", [128, 8], F32, kind="ExternalInput").ap()
    t_id = nc.dram_tensor("ident", [128, 128], F16, kind="ExternalInput").ap()
    t_xp2o = nc.dram_tensor("xp2o", [nsn, 130], F32, kind="ExternalOutput").ap()

    with tile.TileContext(nc) as tc:
        with (
            tc.tile_pool(name="singles", bufs=1) as singles,
            tc.tile_pool(name="gt", bufs=2) as gt,
            tc.tile_pool(name="at", bufs=2) as at,
            tc.tile_pool(name="mt", bufs=2) as mt,
            tc.tile_pool(name="et", bufs=2) as et,
            tc.tile_pool(name="pt", bufs=2) as pt,
            tc.tile_pool(name="asb", bufs=2) as asb,
            tc.tile_pool(name="rr", bufs=4) as rr,
            tc.tile_pool(name="atb", bufs=2) as atb,
            tc.tile_pool(name="e2t", bufs=2) as e2tp,
            tc.tile_pool(name="xsb", bufs=2) as xsb,
            tc.tile_pool(name="aggps", bufs=3, space="PSUM") as aggps,
            tc.tile_pool(name="atps", bufs=2, space="PSUM") as atps,
            tc.tile_pool(name="o1ps", bufs=1, space="PSUM") as o1ps,
            tc.tile_pool(name="x2ps", bufs=1, space="PSUM") as x2ps,
        ):
            w1t_sb = singles.tile([128, 8, 128], F16)
            nc.sync.dma_start(out=w1t_sb, in_=t_w1t)
            w2tv_sb = singles.tile([128, 8, 132], F16)
            nc.sync.dma_start(out=w2tv_sb, in_=t_w2tv)
            b1c_sb = singles.tile([128, 8], F32)
            nc.sync.dma_start(out=b1c_sb, in_=t_b1c)
            id_sb = singles.tile([128, 128], F16)
            nc.sync.dma_start(out=id_sb, in_=t_id)
            mxr_sb = singles.tile([128, 8], F32)
            nc.sync.dma_start(out=mxr_sb, in_=t_mxr)

            ngb = nchunks // GB
            gtiles = [None] * ngb
            ptiles = [None] * ngb

            def issue_batch(gb):
                gsl = slice(gb * GB * 132, (gb + 1) * GB * 132)
                g = gt.tile([128, GB, 132], F16, tag="g")
                nc.sync.dma_start(out=g, in_=t_gx[:, gsl])
                ap_ = at.tile([128, GB, 16], F32, tag="ap")
                asl = slice(gb * GB * 16, (gb + 1) * GB * 16)
                nc.sync.dma_start(out=ap_, in_=t_ap[:, asl])
                mask = mt.tile([128, GB, 16], F16, tag="mask")
                msl = slice(gb * GB * 16, (gb + 1) * GB * 16)
                nc.sync.dma_start(out=mask, in_=t_mask[:, msl])
                # e = leaky(asrc + adst); p = exp(e) * maskexp
                st = et.tile([128, GB, 8], F32, tag="st")
                nc.vector.tensor_tensor(out=st, in0=ap_[:, :, 0:8],
                                        in1=ap_[:, :, 8:16],
                                        op=mybir.AluOpType.add)
                lk = et.tile([128, GB, 8], F32, tag="lk")
                nc.vector.tensor_scalar_mul(lk, st, NEG)
                ee = et.tile([128, GB, 8], F32, tag="ee")
                nc.vector.tensor_tensor(out=ee, in0=st, in1=lk,
                                        op=mybir.AluOpType.max)
                ee2 = et.tile([128, GB, 8], F32, tag="ee2")
                mx_rep = bass.AP(tensor=mxr_sb.tensor, offset=mxr_sb.offset,
                                 ap=[mxr_sb.ap[0], [0, GB], mxr_sb.ap[1]])
                nc.vector.tensor_tensor(out=ee2, in0=ee, in1=mx_rep,
                                        op=mybir.AluOpType.subtract)
                ex = et.tile([128, GB, 8], F16, tag="ex")
                nc.scalar.activation(ex, ee2, mybir.ActivationFunctionType.Exp)
                p = pt.tile([128, GB, 16, 8], F16, tag="p")
                ex_rep = bass.AP(tensor=ex.tensor, offset=ex.offset,
                                 ap=[ex.ap[0], ex.ap[1], [0, 16], ex.ap[2]])
                mask_rep = bass.AP(tensor=mask.tensor, offset=mask.offset,
                                   ap=[mask.ap[0], mask.ap[1], mask.ap[2],
                                       [0, 8]])
                nc.vector.tensor_tensor(out=p, in0=ex_rep, in1=mask_rep,
                                        op=mybir.AluOpType.mult)
                return g, p

            for wb in range(nwb):
                if wb % 2 == 0:
                    gtiles[wb // 2], ptiles[wb // 2] = issue_batch(wb // 2)
                g, p = gtiles[wb // 2], ptiles[wb // 2]
                atb_t = atb.tile([128, WB, 128], F16, tag="atb")
                for half in range(4):
                    aggf = aggps.tile([128, 2, 256], F32, tag="agg")
                    for q in range(2):
                        cb = (wb % 2) * WB + half * 2 + q
                        p_c = p[:, cb, :, :].rearrange("p a b -> p (a b)")
                        nc.tensor.matmul(out=aggf[:, q, 0:129], lhsT=p_c,
                                         rhs=g[:, cb, 0:129],
                                         start=True, stop=True)
                    rc4 = rr.tile([128, 2], F32, tag="rc")
                    nc.vector.reciprocal(rc4, aggf[:, :, 128:129])
                    a4 = asb.tile([128, 2, 128], F16, tag="a")
                    rc4_rep = bass.AP(tensor=rc4.tensor, offset=rc4.offset,
                                      ap=[rc4.ap[0], rc4.ap[1], [0, 128]])
                    nc.vector.tensor_tensor(out=a4, in0=aggf[:, :, 0:128],
                                            in1=rc4_rep,
                                            op=mybir.AluOpType.mult)
                    atpf = atps.tile([128, 2, 128], F16, tag="atp")
                    for q in range(2):
                        nc.tensor.transpose(out=atpf[:, q, :], in_=a4[:, q, :],
                                            identity=id_sb)
                    nc.scalar.activation(
                        atb_t[:, half * 2:(half + 1) * 2, :], atpf,
                        mybir.ActivationFunctionType.Copy)
                # W1 apply + bias + relu -> emb2T; then xp2/a2 contraction
                o1 = o1ps.tile([128, 8, 128], F32, tag="o1")
                atb_r = atb_t.rearrange("p c (n h) -> p h c n", h=8)
                for h in range(8):
                    nc.tensor.matmul(
                        out=o1[:, h, :],
                        lhsT=w1t_sb[:, h, :],
                        rhs=atb_r[:, h, :, :],
                        start=True, stop=True)
                t1 = e2tp.tile([128, 8, 128], F32, tag="t1")
                b1_rep = bass.AP(tensor=b1c_sb.tensor, offset=b1c_sb.offset,
                                 ap=[b1c_sb.ap[0], b1c_sb.ap[1], [0, 128]])
                nc.vector.tensor_tensor(out=t1, in0=o1, in1=b1_rep,
                                        op=mybir.AluOpType.add)
                e2 = e2tp.tile([128, 8, 128], F16, tag="e2")
                nc.vector.tensor_scalar_max(e2, t1, 0.0)
                x2 = x2ps.tile([128, 132], F32, tag="x2")
                for h in range(8):
                    nc.tensor.matmul(out=x2[:, 0:130], lhsT=e2[:, h, :],
                                     rhs=w2tv_sb[:, h, 0:130],
                                     start=(h == 0), stop=(h == 7))
                x2_sb = xsb.tile([128, 130], F32, tag="x2sb")
                nc.scalar.activation(x2_sb, x2[:, 0:130],
                                     mybir.ActivationFunctionType.Copy)
                nc.sync.dma_start(out=t_xp2o[wb * 128:(wb + 1) * 128, :],
                                  in_=x2_sb)
    nc.compile()
    return nc


# ----------------------------------------------------------------------------
# launch C: GAT layer 2 + final linear
# ----------------------------------------------------------------------------

def _build_launch_c(nchunks):
    assert nchunks % GB == 0
    nsn = nchunks * 32

    nc = bacc.Bacc("TRN2", target_bir_lowering=False, debug=False)
    t_gx = nc.dram_tensor("gx2", [128, nchunks * 132], F16,
                          kind="ExternalInput").ap()
    t_ap = nc.dram_tensor("apn2", [128, nchunks * 2], F32,
                          kind="ExternalInput").ap()
    t_mask = nc.dram_tensor("mask012", [128, nchunks * 32], F16,
                            kind="ExternalInput").ap()
    t_mx2 = nc.dram_tensor("mx2c", [128, 1], F32, kind="ExternalInput").ap()
    t_oWT = nc.dram_tensor("outWT", [128, 128], F16, kind="ExternalInput").ap()
    t_ob = nc.dram_tensor("outb", [128, 1], F32, kind="ExternalInput").ap()
    t_b2 = nc.dram_tensor("b2c", [128, 1], F32, kind="ExternalInput").ap()
    t_id = nc.dram_tensor("ident2", [128, 128], F16, kind="ExternalInput").ap()
    t_lgo = nc.dram_tensor("lgo", [128, nsn], F32, kind="ExternalOutput").ap()

    with tile.TileContext(nc) as tc:
        with (
            tc.tile_pool(name="singles", bufs=1) as singles,
            tc.tile_pool(name="gt", bufs=2) as gt,
            tc.tile_pool(name="at", bufs=2) as at,
            tc.tile_pool(name="mt", bufs=2) as mt,
            tc.tile_pool(name="et", bufs=2) as et,
            tc.tile_pool(name="pt", bufs=2) as pt,
            tc.tile_pool(name="asb", bufs=2) as asb,
            tc.tile_pool(name="rr", bufs=4) as rr,
            tc.tile_pool(name="lg", bufs=2) as lgp,
            tc.tile_pool(name="aggps", bufs=4, space="PSUM") as aggps,
            tc.tile_pool(name="atps", bufs=2, space="PSUM") as atps,
            tc.tile_pool(name="lgps", bufs=2, space="PSUM") as lgps,
        ):
            oWT_sb = singles.tile([128, 128], F16)
            nc.sync.dma_start(out=oWT_sb, in_=t_oWT)
            ob_sb = singles.tile([128, 1], F32)
            nc.sync.dma_start(out=ob_sb, in_=t_ob)
            b2_sb = singles.tile([128, 1], F32)
            nc.sync.dma_start(out=b2_sb, in_=t_b2)
            id_sb = singles.tile([128, 128], F16)
            nc.sync.dma_start(out=id_sb, in_=t_id)
            mx2_sb = singles.tile([128, 1], F32)
            nc.sync.dma_start(out=mx2_sb, in_=t_mx2)
            e3t_sb = singles.tile([128, nsn], F16)

            ngb = nchunks // GB
            for gb in range(ngb):
                gsl = slice(gb * GB * 132, (gb + 1) * GB * 132)
                g = gt.tile([128, GB, 132], F16, tag="g")
                nc.sync.dma_start(out=g, in_=t_gx[:, gsl])
                ap_ = at.tile([128, GB, 2], F32, tag="ap")
                asl = slice(gb * GB * 2, (gb + 1) * GB * 2)
                nc.sync.dma_start(out=ap_, in_=t_ap[:, asl])
                mask = mt.tile([128, GB, 32], F16, tag="mask")
                msl = slice(gb * GB * 32, (gb + 1) * GB * 32)
                nc.sync.dma_start(out=mask, in_=t_mask[:, msl])
                st = et.tile([128, GB], F32, tag="st")
                nc.vector.tensor_tensor(out=st, in0=ap_[:, :, 0],
                                        in1=ap_[:, :, 1],
                                        op=mybir.AluOpType.add)
                lk = et.tile([128, GB], F32, tag="lk")
                nc.vector.tensor_scalar_mul(lk, st, NEG)
                ee = et.tile([128, GB], F32, tag="ee")
                nc.vector.tensor_tensor(out=ee, in0=st, in1=lk,
                                        op=mybir.AluOpType.max)
                ee2 = et.tile([128, GB], F32, tag="ee2")
                nc.vector.tensor_scalar_sub(ee2, ee, mx2_sb[:, 0:1])
                ex = et.tile([128, GB], F16, tag="ex")
                nc.scalar.activation(ex, ee2, mybir.ActivationFunctionType.Exp)
                p = pt.tile([128, GB, 32], F16, tag="p")
                ex_rep = bass.AP(tensor=ex.tensor, offset=ex.offset,
                                 ap=[ex.ap[0], ex.ap[1], [0, 32]])
                nc.vector.tensor_tensor(out=p, in0=ex_rep, in1=mask,
                                        op=mybir.AluOpType.mult)

                for grp in range(GB // 4):
                    aggc = aggps.tile([128, 132], F32, tag="agg")
                    for q in range(4):
                        cb = grp * 4 + q
                        nc.tensor.matmul(out=aggc[32 * q:32 * (q + 1), 0:129],
                                         lhsT=p[:, cb, :],
                                         rhs=g[:, cb, 0:129],
                                         start=True, stop=True,
                                         tile_position=(0, 32 * q))
                    rc = rr.tile([128, 1], F32, tag="rc")
                    nc.vector.reciprocal(rc, aggc[:, 128:129])
                    a4 = asb.tile([128, 128], F16, tag="a")
                    nc.scalar.activation(a4, aggc[:, 0:128],
                                         mybir.ActivationFunctionType.Copy,
                                         scale=rc[:, 0:1])
                    atp = atps.tile([128, 128], F16, tag="atp")
                    nc.tensor.transpose(out=atp, in_=a4, identity=id_sb)
                    c0 = (gb * 4 + grp) * 128
                    nc.scalar.activation(e3t_sb[:, c0:c0 + 128], atp,
                                         mybir.ActivationFunctionType.Relu,
                                         bias=b2_sb[:, 0:1])
            # logits^T = outW.T.T @ emb3T + out_b
            nwin = nsn // 512
            for w in range(nwin):
                sl = slice(512 * w, 512 * (w + 1))
                lp = lgps.tile([128, 512], F32, tag="lg")
                nc.tensor.matmul(out=lp, lhsT=oWT_sb, rhs=e3t_sb[:, sl],
                                 start=True, stop=True)
                lsb = lgp.tile([128, 512], F32, tag="lsb")
                nc.vector.tensor_scalar_add(lsb, lp, ob_sb[:, 0:1])
                nc.sync.dma_start(out=t_lgo[:, sl], in_=lsb)
    nc.compile()
    return nc


# ----------------------------------------------------------------------------
# main entry
# ----------------------------------------------------------------------------

def kernel(**inputs):
    cs = np.ascontiguousarray(inputs["constraints_state"], np.float32)
    xs = np.ascontiguousarray(inputs["columns_state"], np.float32)
    node_W = np.asarray(inputs["node_W"], np.float32)
    node_b = np.asarray(inputs["node_b"], np.float32)
    col_W = np.asarray(inputs["col_W"], np.float32)
    col_b = np.asarray(inputs["col_b"], np.float32)
    W1 = np.asarray(inputs["W1"], np.float32)
    att_src1 = np.asarray(inputs["att_src1"], np.float32)
    att_dst1 = np.asarray(inputs["att_dst1"], np.float32)
    b1 = np.asarray(inputs["b1"], np.float32)
    W2 = np.asarray(inputs["W2"], np.float32)
    att_src2 = np.asarray(inputs["att_src2"], np.float32)
    att_dst2 = np.asarray(inputs["att_dst2"], np.float32)
    b2 = np.asarray(inputs["b2"], np.float32)
    out_W = np.asarray(inputs["out_W"], np.float32)
    out_b = np.asarray(inputs["out_b"], np.float32)
    edges = np.asarray(inputs["edges"]).astype(np.int64)

    # ---- weight folds
    W1h = W1.reshape(8, 128, 128)
    vsrc1 = np.einsum("hc,hcd->hd", att_src1, W1h).astype(np.float32)
    vdst1 = np.einsum("hc,hcd->hd", att_dst1, W1h).astype(np.float32)
    w2v = (W2.T @ np.stack([att_src2[0], att_dst2[0]], 1)).astype(np.float32)

    # ---- edges + self loops, per-core packing
    loops = np.arange(N, dtype=np.int64)
    src = np.concatenate([edges[0], loops])
    dst = np.concatenate([edges[1], loops])
    packs1, packs2 = [], []
    for core in range(N_CORES):
        lo, hi = core * SHARD, (core + 1) * SHARD
        packs1.append(_pack_edges(src, dst, lo, hi, 16))
        packs2.append(_pack_edges(src, dst, lo, hi, 32))

    def _roundup(x, m):
        return (x + m - 1) // m * m

    nc1 = _roundup(max(p["n_chunks"] for p in packs1), GB)
    nc2 = _roundup(max(p["n_chunks"] for p in packs2), GB)
    packs1 = [_pad_chunks(p, nc1) for p in packs1]
    packs2 = [_pad_chunks(p, nc2) for p in packs2]

    # ---- compile programs (cached)
    if "a" not in _programs:
        _programs["a"] = _build_launch_a()
    if ("b", nc1) not in _programs:
        _programs[("b", nc1)] = _build_launch_b(nc1)
    if ("c", nc2) not in _programs:
        _programs[("c", nc2)] = _build_launch_c(nc2)
    prog_a = _programs["a"]
    prog_b = _programs[("b", nc1)]
    prog_c = _programs[("c", nc2)]

    # ---- launch A
    vsV = np.concatenate([vsrc1.T, vdst1.T], 1).astype(np.float32)
    in_a = []
    for core in range(N_CORES):
        lo = core * SHARD
        if lo < N_CONS:
            feat = np.tile(cs[lo:lo + SHARD], (1, 2))
            encW = np.concatenate([node_W, np.zeros((128, 8), np.float32)], 1)
            encb_ = node_b
        else:
            feat = np.tile(xs[lo - N_CONS:lo - N_CONS + SHARD], (1, 2))
            encW = col_W
            encb_ = col_b
        encT = np.zeros((16, ENC_COLS), np.float32)
        encT[:feat.shape[1], :SHARD] = feat.T
        in_a.append({
            "encT": encT,
            "encWT": np.ascontiguousarray(encW.T),
            "encb": encb_.reshape(128, 1).astype(np.float32),
            "vsV": vsV,
        })
    res_a = _run(prog_a, in_a, "A")
    emb1 = np.concatenate(
        [res_a.results[c]["embo"][:, :SHARD].T for c in range(N_CORES)], 0)
    a1 = np.concatenate(
        [res_a.results[c]["a1o"][:, :SHARD].T for c in range(N_CORES)], 0)
    a1 = np.ascontiguousarray(a1, np.float32)               # [N, 16]

    # ---- host: expanded per-slot inputs for launch B
    emb1e = np.zeros((N, 132), np.float16)
    emb1e[:, 0:128] = emb1.astype(np.float16)
    emb1e[:, 128] = 1.0
    mx1 = _leaky_np(a1[:, 0:8].max(0) + a1[:, 8:16].max(0))

    ident = np.eye(128, dtype=np.float16)
    w1t = np.ascontiguousarray(W1h.transpose(2, 0, 1), np.float16)
    w2tv = np.zeros((128, 8, 132), np.float16)
    w2tv[:, :, 0:128] = W2.T.reshape(8, 128, 128).transpose(1, 0, 2)
    w2tv[:, :, 128:130] = w2v.reshape(8, 128, 2).transpose(1, 0, 2)
    b1c = np.ascontiguousarray(b1.reshape(8, 128).T, np.float32)

    in_b = []
    for core in range(N_CORES):
        pk = packs1[core]
        nc_ = pk["n_chunks"]
        apn = np.concatenate([
            a1[pk["src_idx"], 0:8], a1[pk["dst_idx"], 8:16]], 1)
        apn = np.ascontiguousarray(
            apn.reshape(nc_, 128, 16).transpose(1, 0, 2).reshape(128, -1),
            np.float32)
        in_b.append({
            "gx": _expand_slots(pk, emb1e, np.float16),
            "apn": apn,
            "mask01": _mask01(pk, np.float16),
            "mxr": np.tile(mx1, (128, 1)).astype(np.float32),
            "w1t": w1t, "w2tv": w2tv, "b1c": b1c, "ident": ident,
        })
    res_b = _run(prog_b, in_b, "B")

    # ---- host: assemble xp2 / a2 tables
    tab2e = np.zeros((N, 132), np.float16)
    tab2e[:, 128] = 1.0
    a2 = np.zeros((N, 2), np.float32)
    for core in range(N_CORES):
        nm = packs1[core]["node_map"]
        valid = nm >= 0
        xo = res_b.results[core]["xp2o"]
        tab2e[nm[valid], 0:128] = xo[valid, 0:128].astype(np.float16)
        a2[nm[valid]] = xo[valid, 128:130]
    mx2 = _leaky_np(np.array(
        [a2[:, 0].max() + a2[:, 1].max()], np.float32))

    in_c = []
    for core in range(N_CORES):
        pk = packs2[core]
        nc_ = pk["n_chunks"]
        apn2 = np.stack([a2[pk["src_idx"], 0], a2[pk["dst_idx"], 1]], 1)
        apn2 = np.ascontiguousarray(
            apn2.reshape(nc_, 128, 2).transpose(1, 0, 2).reshape(128, -1),
            np.float32)
        in_c.append({
            "gx2": _expand_slots(pk, tab2e, np.float16),
            "apn2": apn2,
            "mask012": _mask01(pk, np.float16),
            "mx2c": np.full((128, 1), mx2[0], np.float32),
            "outWT": np.ascontiguousarray(out_W.T, np.float16),
            "outb": out_b.reshape(128, 1).astype(np.float32),
            "b2c": b2.reshape(128, 1).astype(np.float32),
            "ident2": ident,
        })
    res_c = _run(prog_c, in_c, "C")

    logits = np.zeros((N, 128), np.float32)
    for core in range(N_CORES):
        nm = packs2[core]["node_map"]
        valid = nm >= 0
        logits[nm[valid]] = res_c.results[core]["lgo"][:, valid].T

    return logits[-N_COLS:].astype(np.float32)


_trace = {"enable": False, "dir": None, "exec_ns": {}}


def _run(prog, in_maps, tag):
    kwargs = {}
    if _trace["enable"]:
        import os
        d = os.path.join(_trace["dir"], tag)
        os.makedirs(d, exist_ok=True)
        kwargs = dict(trace=True, tmpdir=d)
    res = run_bass_kernel_spmd(prog, in_maps, core_ids=list(range(N_CORES)),
                               **kwargs)
    _trace["exec_ns"][tag] = res.exec_time_ns
    return res


# revision 16
# speedup vs baseline: 1.0516x; 1.0516x over previous
"""Trainium2 Bass kernel for nn_GAT_66821101191795 (2-layer GAT, 8 NeuronCores).

Strategy (graph/data parallel, dst-sharded):
- Host: add self loops, sort edges by dst, shard dst nodes into 8 blocks of
  2500, pack each destination node's edges into contiguous slots of 128-slot
  chunks (<=16 dst nodes per chunk for layer 1, <=32 for layer 2). Per-edge
  source features are gathered host-side ("all-to-all the gathered source
  features") into per-slot fp16 tiles; attention a-values are likewise
  expanded per slot. Weight reparameterisations: vsrc/vdst = att @ W folds so
  attention logits come from emb directly; W2.T @ att2 folds the layer-2
  attention projections.
- Launch A (device): node/col encoders -> emb1^T shard + a1^T shard per core.
- Launch B (device): layer-1 attention (leaky+exp+softmax via masked
  numerators and a ones-matmul for denominators), aggregation as one
  128x128x128 fp16 matmul per chunk, PE transpose, W1 apply + bias + relu,
  xp2 = emb2 @ W2.T and a2 = emb2 @ w2v contractions.
- Launch C (device): layer-2 attention + aggregation (+b2, relu) + final
  linear -> logits^T slots. Host unpacks slots -> logits [10000, 128].
"""

import sys

for _p in ("/opt/trn_rl_repo", "/root/.axon_site"):
    if _p not in sys.path:
        sys.path.insert(0, _p)

import numpy as np

import concourse.bacc as bacc
import concourse.bass as bass
import concourse.tile as tile
from concourse import mybir
from concourse.bass_utils import run_bass_kernel_spmd

F32 = mybir.dt.float32
F16 = mybir.dt.float16

N_CONS = 10000
N_COLS = 10000
N = N_CONS + N_COLS
N_CORES = 8
SHARD = N // N_CORES
NEG = 0.2
GB = 16            # chunks per compute batch
WB = 8             # chunks per W1 batch (launch B)
ENC_COLS = 2560    # padded shard width for launch A (5 x 512)

_programs = {}


# ----------------------------------------------------------------------------
# host-side edge preprocessing
# ----------------------------------------------------------------------------

def _pack_edges(src, dst, lo, hi, max_nodes):
    """Pack edges with dst in [lo, hi) into 128-slot chunks.

    Each dst node's edges occupy contiguous slots within a single chunk; at
    most max_nodes nodes per chunk.
    """
    sel = (dst >= lo) & (dst < hi)
    s = src[sel]
    d = dst[sel]
    order = np.argsort(d, kind="stable")
    s = s[order]
    d = d[order]
    nodes, counts = np.unique(d, return_counts=True)
    assert counts.max() <= 128, f"degree {counts.max()} > 128 unsupported"
    offs = np.concatenate([[0], np.cumsum(counts)])

    chunk_src = []
    chunk_nodes = []
    cur_src, cur_nodes, cur_slots = [], [], 0
    for i in range(len(nodes)):
        k = int(counts[i])
        if cur_slots + k > 128 or len(cur_nodes) >= max_nodes:
            chunk_src.append(cur_src)
            chunk_nodes.append(cur_nodes)
            cur_src, cur_nodes, cur_slots = [], [], 0
        cur_src.append(s[offs[i]:offs[i + 1]])
        cur_nodes.append((int(nodes[i]), k))
        cur_slots += k
    if cur_nodes:
        chunk_src.append(cur_src)
        chunk_nodes.append(cur_nodes)

    nc_ = len(chunk_nodes)
    src_idx = np.zeros(128 * nc_, np.int64)
    dst_idx = np.zeros(128 * nc_, np.int64)
    node_col = np.full(128 * nc_, -1, np.int32)
    node_map = np.full(nc_ * max_nodes, -1, np.int32)
    for c in range(nc_):
        slot = 0
        for j, (nd, k) in enumerate(chunk_nodes[c]):
            sl = slice(128 * c + slot, 128 * c + slot + k)
            src_idx[sl] = chunk_src[c][j]
            dst_idx[sl] = nd
            node_col[sl] = j
            node_map[c * max_nodes + j] = nd
            slot += k
    return dict(n_chunks=nc_, src_idx=src_idx, dst_idx=dst_idx,
                node_col=node_col, node_map=node_map, max_nodes=max_nodes)


def _pad_chunks(pk, n_chunks_to):
    nc_, mx = pk["n_chunks"], pk["max_nodes"]
    pad = n_chunks_to - nc_
    assert pad >= 0
    if pad:
        z = np.zeros(128 * pad, np.int64)
        pk["src_idx"] = np.concatenate([pk["src_idx"], z])
        pk["dst_idx"] = np.concatenate([pk["dst_idx"], z])
        pk["node_col"] = np.concatenate(
            [pk["node_col"], np.full(128 * pad, -1, np.int32)])
        pk["node_map"] = np.concatenate(
            [pk["node_map"], np.full(mx * pad, -1, np.int32)])
    pk["n_chunks"] = n_chunks_to
    return pk


def _expand_slots(pk, table, dtype):
    """Per-slot rows table[src_idx] laid out [128, nc * width]."""
    nc_ = pk["n_chunks"]
    w = table.shape[1]
    t = table[pk["src_idx"]].reshape(nc_, 128, w).transpose(1, 0, 2)
    return np.ascontiguousarray(t.reshape(128, nc_ * w), dtype)


def _mask01(pk, dtype):
    """indicator mask [128, nc*max_nodes]: 1.0 at the slot's node col."""
    nc_, mx = pk["n_chunks"], pk["max_nodes"]
    ncol = pk["node_col"].reshape(nc_, 128)
    cols = np.arange(mx)
    m = (ncol[:, :, None] == cols[None, None, :]).astype(np.float32)
    out = m.transpose(1, 0, 2).reshape(128, nc_ * mx)
    return np.ascontiguousarray(out, dtype)


def _leaky_np(x):
    return np.where(x > 0, x, NEG * x).astype(np.float32)


# ----------------------------------------------------------------------------
# launch A: encoders
# ----------------------------------------------------------------------------

def _build_launch_a():
    nc = bacc.Bacc("TRN2", target_bir_lowering=False, debug=False)
    encT = nc.dram_tensor("encT", [16, ENC_COLS], F32, kind="ExternalInput").ap()
    encWT = nc.dram_tensor("encWT", [16, 128], F32, kind="ExternalInput").ap()
    encb = nc.dram_tensor("encb", [128, 1], F32, kind="ExternalInput").ap()
    vsV = nc.dram_tensor("vsV", [128, 16], F32, kind="ExternalInput").ap()
    embo = nc.dram_tensor("embo", [128, ENC_COLS], F32, kind="ExternalOutput").ap()
    a1o = nc.dram_tensor("a1o", [16, ENC_COLS], F32, kind="ExternalOutput").ap()

    with tile.TileContext(nc) as tc:
        with (
            tc.tile_pool(name="singles", bufs=1) as singles,
            tc.tile_pool(name="ps1", bufs=2, space="PSUM") as ps1,
            tc.tile_pool(name="ps2", bufs=2, space="PSUM") as ps2,
        ):
            encT_sb = singles.tile([16, ENC_COLS], F32)
            nc.sync.dma_start(out=encT_sb, in_=encT)
            encWT_sb = singles.tile([16, 128], F32)
            nc.sync.dma_start(out=encWT_sb, in_=encWT)
            encb_sb = singles.tile([128, 1], F32)
            nc.sync.dma_start(out=encb_sb, in_=encb)
            vsV_sb = singles.tile([128, 16], F32)
            nc.sync.dma_start(out=vsV_sb, in_=vsV)
            emb_sb = singles.tile([128, ENC_COLS], F32)
            a1_sb = singles.tile([16, ENC_COLS], F32)

            nw = ENC_COLS // 512
            for w in range(nw):
                sl = slice(512 * w, 512 * (w + 1))
                p1 = ps1.tile([128, 512], F32)
                nc.tensor.matmul(out=p1, lhsT=encWT_sb, rhs=encT_sb[:, sl],
                                 start=True, stop=True)
                nc.scalar.activation(emb_sb[:, sl], p1,
                                     mybir.ActivationFunctionType.Relu,
                                     bias=encb_sb[:, 0:1])
            for w in range(nw):
                sl = slice(512 * w, 512 * (w + 1))
                p2 = ps2.tile([16, 512], F32)
                nc.tensor.matmul(out=p2, lhsT=vsV_sb, rhs=emb_sb[:, sl],
                                 start=True, stop=True)
                nc.vector.tensor_copy(a1_sb[:, sl], p2)
            nc.sync.dma_start(out=embo, in_=emb_sb)
            nc.sync.dma_start(out=a1o, in_=a1_sb)
    nc.compile()
    return nc


# ----------------------------------------------------------------------------
# launch B: GAT layer 1 (+ W1, relu, xp2, a2)
# ----------------------------------------------------------------------------

def _build_launch_b(nchunks):
    assert nchunks % GB == 0
    nsn = nchunks * 16
    nwb = nchunks // WB

    nc = bacc.Bacc("TRN2", target_bir_lowering=False, debug=False)
    t_gx = nc.dram_tensor("gx", [128, nchunks * 132], F16,
                          kind="ExternalInput").ap()
    t_ap = nc.dram_tensor("apn", [128, nchunks * 16], F32,
                          kind="ExternalInput").ap()
    t_mask = nc.dram_tensor("mask01", [128, nchunks * 16], F16,
                            kind="ExternalInput").ap()
    t_mxr = nc.dram_tensor("mxr", [128, 8], F32, kind="ExternalInput").ap()
    t_w1t = nc.dram_tensor("w1t", [128, 8, 128], F16, kind="ExternalInput").ap()
    t_w2tv = nc.dram_tensor("w2tv", [128, 8, 132], F16,
                            kind="ExternalInput").ap()
    t_b1c = nc.dram_tensor("b1c", [128, 8], F32, kind="ExternalInput").ap()
    t_id = nc.dram_tensor("ident", [128, 128], F16, kind="ExternalInput").ap()
    t_xp2o = nc.dram_tensor("xp2o", [nsn, 130], F32, kind="ExternalOutput").ap()

    with tile.TileContext(nc) as tc:
        with (
            tc.tile_pool(name="singles", bufs=1) as singles,
            tc.tile_pool(name="gt", bufs=2) as gt,
            tc.tile_pool(name="at", bufs=2) as at,
            tc.tile_pool(name="mt", bufs=2) as mt,
            tc.tile_pool(name="et", bufs=2) as et,
            tc.tile_pool(name="pt", bufs=2) as pt,
            tc.tile_pool(name="asb", bufs=2) as asb,
            tc.tile_pool(name="rr", bufs=4) as rr,
            tc.tile_pool(name="atb", bufs=2) as atb,
            tc.tile_pool(name="e2t", bufs=2) as e2tp,
            tc.tile_pool(name="xsb", bufs=2) as xsb,
            tc.tile_pool(name="aggps", bufs=2, space="PSUM") as aggps,
            tc.tile_pool(name="atps", bufs=1, space="PSUM") as atps,
            tc.tile_pool(name="o1ps", bufs=1, space="PSUM") as o1ps,
            tc.tile_pool(name="x2ps", bufs=1, space="PSUM") as x2ps,
        ):
            w1t_sb = singles.tile([128, 8, 128], F16)
            nc.sync.dma_start(out=w1t_sb, in_=t_w1t)
            w2tv_sb = singles.tile([128, 8, 132], F16)
            nc.sync.dma_start(out=w2tv_sb, in_=t_w2tv)
            b1c_sb = singles.tile([128, 8], F32)
            nc.sync.dma_start(out=b1c_sb, in_=t_b1c)
            id_sb = singles.tile([128, 128], F16)
            nc.sync.dma_start(out=id_sb, in_=t_id)
            mxr_sb = singles.tile([128, 8], F32)
            nc.sync.dma_start(out=mxr_sb, in_=t_mxr)

            ngb = nchunks // GB
            gtiles = [None] * ngb
            ptiles = [None] * ngb

            def issue_batch(gb):
                gsl = slice(gb * GB * 132, (gb + 1) * GB * 132)
                g = gt.tile([128, GB, 132], F16, tag="g")
                nc.sync.dma_start(out=g, in_=t_gx[:, gsl])
                ap_ = at.tile([128, GB, 16], F32, tag="ap")
                asl = slice(gb * GB * 16, (gb + 1) * GB * 16)
                nc.sync.dma_start(out=ap_, in_=t_ap[:, asl])
                mask = mt.tile([128, GB, 16], F16, tag="mask")
                msl = slice(gb * GB * 16, (gb + 1) * GB * 16)
                nc.sync.dma_start(out=mask, in_=t_mask[:, msl])
                # e = leaky(asrc + adst); p = exp(e) * maskexp
                st = et.tile([128, GB, 8], F32, tag="st")
                nc.vector.tensor_tensor(out=st, in0=ap_[:, :, 0:8],
                                        in1=ap_[:, :, 8:16],
                                        op=mybir.AluOpType.add)
                lk = et.tile([128, GB, 8], F32, tag="lk")
                nc.vector.tensor_scalar_mul(lk, st, NEG)
                ee = et.tile([128, GB, 8], F32, tag="ee")
                nc.vector.tensor_tensor(out=ee, in0=st, in1=lk,
                                        op=mybir.AluOpType.max)
                ee2 = et.tile([128, GB, 8], F32, tag="ee2")
                mx_rep = bass.AP(tensor=mxr_sb.tensor, offset=mxr_sb.offset,
                                 ap=[mxr_sb.ap[0], [0, GB], mxr_sb.ap[1]])
                nc.vector.tensor_tensor(out=ee2, in0=ee, in1=mx_rep,
                                        op=mybir.AluOpType.subtract)
                ex = et.tile([128, GB, 8], F16, tag="ex")
                nc.scalar.activation(ex, ee2, mybir.ActivationFunctionType.Exp)
                p = pt.tile([128, GB, 16, 8], F16, tag="p")
                ex_rep = bass.AP(tensor=ex.tensor, offset=ex.offset,
                                 ap=[ex.ap[0], ex.ap[1], [0, 16], ex.ap[2]])
                mask_rep = bass.AP(tensor=mask.tensor, offset=mask.offset,
                                   ap=[mask.ap[0], mask.ap[1], mask.ap[2],
                                       [0, 8]])
                nc.vector.tensor_tensor(out=p, in0=ex_rep, in1=mask_rep,
                                        op=mybir.AluOpType.mult)
                return g, p

            for wb in range(nwb):
                if wb % 2 == 0:
                    gtiles[wb // 2], ptiles[wb // 2] = issue_batch(wb // 2)
                g, p = gtiles[wb // 2], ptiles[wb // 2]
                atb_t = atb.tile([128, WB, 128], F16, tag="atb")
                for half in range(2):
                    aggf = aggps.tile([128, 4, 256], F32, tag="agg")
                    for q in range(4):
                        cb = (wb % 2) * WB + half * 4 + q
                        p_c = p[:, cb, :, :].rearrange("p a b -> p (a b)")
                        nc.tensor.matmul(out=aggf[:, q, 0:129], lhsT=p_c,
                                         rhs=g[:, cb, 0:129],
                                         start=True, stop=True)
                    rc4 = rr.tile([128, 4], F32, tag="rc")
                    nc.vector.reciprocal(rc4, aggf[:, :, 128:129])
                    a4 = asb.tile([128, 4, 128], F16, tag="a")
                    rc4_rep = bass.AP(tensor=rc4.tensor, offset=rc4.offset,
                                      ap=[rc4.ap[0], rc4.ap[1], [0, 128]])
                    nc.vector.tensor_tensor(out=a4, in0=aggf[:, :, 0:128],
                                            in1=rc4_rep,
                                            op=mybir.AluOpType.mult)
                    atpf = atps.tile([128, 4, 128], F16, tag="atp")
                    for q in range(4):
                        nc.tensor.transpose(out=atpf[:, q, :], in_=a4[:, q, :],
                                            identity=id_sb)
                    nc.scalar.activation(
                        atb_t[:, half * 4:(half + 1) * 4, :], atpf,
                        mybir.ActivationFunctionType.Copy)
                # W1 apply + bias + relu -> emb2T; then xp2/a2 contraction
                o1 = o1ps.tile([128, 8, 128], F32, tag="o1")
                atb_r = atb_t.rearrange("p c (n h) -> p h c n", h=8)
                for h in range(8):
                    nc.tensor.matmul(
                        out=o1[:, h, :],
                        lhsT=w1t_sb[:, h, :],
                        rhs=atb_r[:, h, :, :],
                        start=True, stop=True)
                t1 = e2tp.tile([128, 8, 128], F32, tag="t1")
                b1_rep = bass.AP(tensor=b1c_sb.tensor, offset=b1c_sb.offset,
                                 ap=[b1c_sb.ap[0], b1c_sb.ap[1], [0, 128]])
                nc.vector.tensor_tensor(out=t1, in0=o1, in1=b1_rep,
                                        op=mybir.AluOpType.add)
                e2 = e2tp.tile([128, 8, 128], F16, tag="e2")
                nc.vector.tensor_scalar_max(e2, t1, 0.0)
                x2 = x2ps.tile([128, 132], F32, tag="x2")
                for h in range(8):
                    nc.tensor.matmul(out=x2[:, 0:130], lhsT=e2[:, h, :],
                                     rhs=w2tv_sb[:, h, 0:130],
                                     start=(h == 0), stop=(h == 7))
                x2_sb = xsb.tile([128, 130], F32, tag="x2sb")
                nc.scalar.activation(x2_sb, x2[:, 0:130],
                                     mybir.ActivationFunctionType.Copy)
                nc.sync.dma_start(out=t_xp2o[wb * 128:(wb + 1) * 128, :],
                                  in_=x2_sb)
    nc.compile()
    return nc


# ----------------------------------------------------------------------------
# launch C: GAT layer 2 + final linear
# ----------------------------------------------------------------------------

def _build_launch_c(nchunks):
    assert nchunks % GB == 0
    nsn = nchunks * 32

    nc = bacc.Bacc("TRN2", target_bir_lowering=False, debug=False)
    t_gx = nc.dram_tensor("gx2", [128, nchunks * 132], F16,
                          kind="ExternalInput").ap()
    t_ap = nc.dram_tensor("apn2", [128, nchunks * 2], F32,
                          kind="ExternalInput").ap()
    t_mask = nc.dram_tensor("mask012", [128, nchunks * 32], F16,
                            kind="ExternalInput").ap()
    t_mx2 = nc.dram_tensor("mx2c", [128, 1], F32, kind="ExternalInput").ap()
    t_oWT = nc.dram_tensor("outWT", [128, 128], F16, kind="ExternalInput").ap()
    t_ob = nc.dram_tensor("outb", [128, 1], F32, kind="ExternalInput").ap()
    t_b2 = nc.dram_tensor("b2c", [128, 1], F32, kind="ExternalInput").ap()
    t_id = nc.dram_tensor("ident2", [128, 128], F16, kind="ExternalInput").ap()
    t_lgo = nc.dram_tensor("lgo", [128, nsn], F32, kind="ExternalOutput").ap()

    with tile.TileContext(nc) as tc:
        with (
            tc.tile_pool(name="singles", bufs=1) as singles,
            tc.tile_pool(name="gt", bufs=2) as gt,
            tc.tile_pool(name="at", bufs=2) as at,
            tc.tile_pool(name="mt", bufs=2) as mt,
            tc.tile_pool(name="et", bufs=2) as et,
            tc.tile_pool(name="pt", bufs=2) as pt,
            tc.tile_pool(name="asb", bufs=2) as asb,
            tc.tile_pool(name="rr", bufs=4) as rr,
            tc.tile_pool(name="lg", bufs=2) as lgp,
            tc.tile_pool(name="aggps", bufs=4, space="PSUM") as aggps,
            tc.tile_pool(name="atps", bufs=2, space="PSUM") as atps,
            tc.tile_pool(name="lgps", bufs=2, space="PSUM") as lgps,
        ):
            oWT_sb = singles.tile([128, 128], F16)
            nc.sync.dma_start(out=oWT_sb, in_=t_oWT)
            ob_sb = singles.tile([128, 1], F32)
            nc.sync.dma_start(out=ob_sb, in_=t_ob)
            b2_sb = singles.tile([128, 1], F32)
            nc.sync.dma_start(out=b2_sb, in_=t_b2)
            id_sb = singles.tile([128, 128], F16)
            nc.sync.dma_start(out=id_sb, in_=t_id)
            mx2_sb = singles.tile([128, 1], F32)
            nc.sync.dma_start(out=mx2_sb, in_=t_mx2)
            e3t_sb = singles.tile([128, nsn], F16)

            ngb = nchunks // GB
            for gb in range(ngb):
                gsl = slice(gb * GB * 132, (gb + 1) * GB * 132)
                g = gt.tile([128, GB, 132], F16, tag="g")
                nc.sync.dma_start(out=g, in_=t_gx[:, gsl])
                ap_ = at.tile([128, GB, 2], F32, tag="ap")
                asl = slice(gb * GB * 2, (gb + 1) * GB * 2)
                nc.sync.dma_start(out=ap_, in_=t_ap[:, asl])
                mask = mt.tile([128, GB, 32], F16, tag="mask")
                msl = slice(gb * GB * 32, (gb + 1) * GB * 32)
                nc.sync.dma_start(out=mask, in_=t_mask[:, msl])
                st = et.tile([128, GB], F32, tag="st")
                nc.vector.tensor_tensor(out=st, in0=ap_[:, :, 0],
                                        in1=ap_[:, :, 1],
                                        op=mybir.AluOpType.add)
                lk = et.tile([128, GB], F32, tag="lk")
                nc.vector.tensor_scalar_mul(lk, st, NEG)
                ee = et.tile([128, GB], F32, tag="ee")
                nc.vector.tensor_tensor(out=ee, in0=st, in1=lk,
                                        op=mybir.AluOpType.max)
                ee2 = et.tile([128, GB], F32, tag="ee2")
                nc.vector.tensor_scalar_sub(ee2, ee, mx2_sb[:, 0:1])
                ex = et.tile([128, GB], F16, tag="ex")
                nc.scalar.activation(ex, ee2, mybir.ActivationFunctionType.Exp)
                p = pt.tile([128, GB, 32], F16, tag="p")
                ex_rep = bass.AP(tensor=ex.tensor, offset=ex.offset,
                                 ap=[ex.ap[0], ex.ap[1], [0, 32]])
                nc.vector.tensor_tensor(out=p, in0=ex_rep, in1=mask,
                                        op=mybir.AluOpType.mult)

                for grp in range(GB // 4):
                    aggc = aggps.tile([128, 132], F32, tag="agg")
                    for q in range(4):
                        cb = grp * 4 + q
                        nc.tensor.matmul(out=aggc[32 * q:32 * (q + 1), 0:129],
                                         lhsT=p[:, cb, :],
                                         rhs=g[:, cb, 0:129],
                                         start=True, stop=True,
                                         tile_position=(0, 32 * q))
                    rc = rr.tile([128, 1], F32, tag="rc")
                    nc.vector.reciprocal(rc, aggc[:, 128:129])
                    a4 = asb.tile([128, 128], F16, tag="a")
                    nc.scalar.activation(a4, aggc[:, 0:128],
                                         mybir.ActivationFunctionType.Copy,
                                         scale=rc[:, 0:1])
                    atp = atps.tile([128, 128], F16, tag="atp")
                    nc.tensor.transpose(out=atp, in_=a4, identity=id_sb)
                    c0 = (gb * 4 + grp) * 128
                    nc.scalar.activation(e3t_sb[:, c0:c0 + 128], atp,
                                         mybir.ActivationFunctionType.Relu,
                                         bias=b2_sb[:, 0:1])
            # logits^T = outW.T.T @ emb3T + out_b
            nwin = nsn // 512
            for w in range(nwin):
                sl = slice(512 * w, 512 * (w + 1))
                lp = lgps.tile([128, 512], F32, tag="lg")
                nc.tensor.matmul(out=lp, lhsT=oWT_sb, rhs=e3t_sb[:, sl],
                                 start=True, stop=True)
                lsb = lgp.tile([128, 512], F32, tag="lsb")
                nc.vector.tensor_scalar_add(lsb, lp, ob_sb[:, 0:1])
                nc.sync.dma_start(out=t_lgo[:, sl], in_=lsb)
    nc.compile()
    return nc


# ----------------------------------------------------------------------------
# main entry
# ----------------------------------------------------------------------------

def kernel(**inputs):
    cs = np.ascontiguousarray(inputs["constraints_state"], np.float32)
    xs = np.ascontiguousarray(inputs["columns_state"], np.float32)
    node_W = np.asarray(inputs["node_W"], np.float32)
    node_b = np.asarray(inputs["node_b"], np.float32)
    col_W = np.asarray(inputs["col_W"], np.float32)
    col_b = np.asarray(inputs["col_b"], np.float32)
    W1 = np.asarray(inputs["W1"], np.float32)
    att_src1 = np.asarray(inputs["att_src1"], np.float32)
    att_dst1 = np.asarray(inputs["att_dst1"], np.float32)
    b1 = np.asarray(inputs["b1"], np.float32)
    W2 = np.asarray(inputs["W2"], np.float32)
    att_src2 = np.asarray(inputs["att_src2"], np.float32)
    att_dst2 = np.asarray(inputs["att_dst2"], np.float32)
    b2 = np.asarray(inputs["b2"], np.float32)
    out_W = np.asarray(inputs["out_W"], np.float32)
    out_b = np.asarray(inputs["out_b"], np.float32)
    edges = np.asarray(inputs["edges"]).astype(np.int64)

    # ---- weight folds
    W1h = W1.reshape(8, 128, 128)
    vsrc1 = np.einsum("hc,hcd->hd", att_src1, W1h).astype(np.float32)
    vdst1 = np.einsum("hc,hcd->hd", att_dst1, W1h).astype(np.float32)
    w2v = (W2.T @ np.stack([att_src2[0], att_dst2[0]], 1)).astype(np.float32)

    # ---- edges + self loops, per-core packing
    loops = np.arange(N, dtype=np.int64)
    src = np.concatenate([edges[0], loops])
    dst = np.concatenate([edges[1], loops])
    packs1, packs2 = [], []
    for core in range(N_CORES):
        lo, hi = core * SHARD, (core + 1) * SHARD
        packs1.append(_pack_edges(src, dst, lo, hi, 16))
        packs2.append(_pack_edges(src, dst, lo, hi, 32))

    def _roundup(x, m):
        return (x + m - 1) // m * m

    nc1 = _roundup(max(p["n_chunks"] for p in packs1), GB)
    nc2 = _roundup(max(p["n_chunks"] for p in packs2), GB)
    packs1 = [_pad_chunks(p, nc1) for p in packs1]
    packs2 = [_pad_chunks(p, nc2) for p in packs2]

    # ---- compile programs (cached)
    if "a" not in _programs:
        _programs["a"] = _build_launch_a()
    if ("b", nc1) not in _programs:
        _programs[("b", nc1)] = _build_launch_b(nc1)
    if ("c", nc2) not in _programs:
        _programs[("c", nc2)] = _build_launch_c(nc2)
    prog_a = _programs["a"]
    prog_b = _programs[("b", nc1)]
    prog_c = _programs[("c", nc2)]

    # ---- launch A
    vsV = np.concatenate([vsrc1.T, vdst1.T], 1).astype(np.float32)
    in_a = []
    for core in range(N_CORES):
        lo = core * SHARD
        if lo < N_CONS:
            feat = np.tile(cs[lo:lo + SHARD], (1, 2))
            encW = np.concatenate([node_W, np.zeros((128, 8), np.float32)], 1)
            encb_ = node_b
        else:
            feat = np.tile(xs[lo - N_CONS:lo - N_CONS + SHARD], (1, 2))
            encW = col_W
            encb_ = col_b
        encT = np.zeros((16, ENC_COLS), np.float32)
        encT[:feat.shape[1], :SHARD] = feat.T
        in_a.append({
            "encT": encT,
            "encWT": np.ascontiguousarray(encW.T),
            "encb": encb_.reshape(128, 1).astype(np.float32),
            "vsV": vsV,
        })
    res_a = _run(prog_a, in_a, "A")
    emb1 = np.concatenate(
        [res_a.results[c]["embo"][:, :SHARD].T for c in range(N_CORES)], 0)
    a1 = np.concatenate(
        [res_a.results[c]["a1o"][:, :SHARD].T for c in range(N_CORES)], 0)
    a1 = np.ascontiguousarray(a1, np.float32)               # [N, 16]

    # ---- host: expanded per-slot inputs for launch B
    emb1e = np.zeros((N, 132), np.float16)
    emb1e[:, 0:128] = emb1.astype(np.float16)
    emb1e[:, 128] = 1.0
    mx1 = _leaky_np(a1[:, 0:8].max(0) + a1[:, 8:16].max(0))

    ident = np.eye(128, dtype=np.float16)
    w1t = np.ascontiguousarray(W1h.transpose(2, 0, 1), np.float16)
    w2tv = np.zeros((128, 8, 132), np.float16)
    w2tv[:, :, 0:128] = W2.T.reshape(8, 128, 128).transpose(1, 0, 2)
    w2tv[:, :, 128:130] = w2v.reshape(8, 128, 2).transpose(1, 0, 2)
    b1c = np.ascontiguousarray(b1.reshape(8, 128).T, np.float32)

    in_b = []
    for core in range(N_CORES):
        pk = packs1[core]
        nc_ = pk["n_chunks"]
        apn = np.concatenate([
            a1[pk["src_idx"], 0:8], a1[pk["dst_idx"], 8:16]], 1)
        apn = np.ascontiguousarray(
            apn.reshape(nc_, 128, 16).transpose(1, 0, 2).reshape(128, -1),
            np.float32)
        in_b.append({
            "gx": _expand_slots(pk, emb1e, np.float16),
            "apn": apn,
            "mask01": _mask01(pk, np.float16),
            "mxr": np.tile(mx1, (128, 1)).astype(np.float32),
            "w1t": w1t, "w2tv": w2tv, "b1c": b1c, "ident": ident,
        })
    res_b = _run(prog_b, in_b, "B")

    # ---- host: assemble xp2 / a2 tables
    tab2e = np.zeros((N, 132), np.float16)
    tab2e[:, 128] = 1.0
    a2 = np.zeros((N, 2), np.float32)
    for core in range(N_CORES):
        nm = packs1[core]["node_map"]
        valid = nm >= 0
        xo = res_b.results[core]["xp2o"]
        tab2e[nm[valid], 0:128] = xo[valid, 0:128].astype(np.float16)
        a2[nm[valid]] = xo[valid, 128:130]
    mx2 = _leaky_np(np.array(
        [a2[:, 0].max() + a2[:, 1].max()], np.float32))

    in_c = []
    for core in range(N_CORES):
        pk = packs2[core]
        nc_ = pk["n_chunks"]
        apn2 = np.stack([a2[pk["src_idx"], 0], a2[pk["dst_idx"], 1]], 1)
        apn2 = np.ascontiguousarray(
            apn2.reshape(nc_, 128, 2).transpose(1, 0, 2).reshape(128, -1),
            np.float32)
        in_c.append({
            "gx2": _expand_slots(pk, tab2e, np.float16),
            "apn2": apn2,
            "mask012": _mask01(pk, np.float16),
            "mx2c": np.full((128, 1), mx2[0], np.float32),
            "outWT": np.ascontiguousarray(out_W.T, np.float16),
            "outb": out_b.reshape(128, 1).astype(np.float32),
            "b2c": b2.reshape(128, 1).astype(np.float32),
            "ident2": ident,
        })
    res_c = _run(prog_c, in_c, "C")

    logits = np.zeros((N, 128), np.float32)
    for core in range(N_CORES):
        nm = packs2[core]["node_map"]
        valid = nm >= 0
        logits[nm[valid]] = res_c.results[core]["lgo"][:, valid].T

    return logits[-N_COLS:].astype(np.float32)


_trace = {"enable": False, "dir": None, "exec_ns": {}}


def _run(prog, in_maps, tag):
    kwargs = {}
    if _trace["enable"]:
        import os
        d = os.path.join(_trace["dir"], tag)
        os.makedirs(d, exist_ok=True)
        kwargs = dict(trace=True, tmpdir=d)
    res = run_bass_kernel_spmd(prog, in_maps, core_ids=list(range(N_CORES)),
                               **kwargs)
    _trace["exec_ns"][tag] = res.exec_time_ns
    return res


# revision 17
# speedup vs baseline: 1.0720x; 1.0195x over previous
"""Trainium2 Bass kernel for nn_GAT_66821101191795 (2-layer GAT, 8 NeuronCores).

Strategy (graph/data parallel, dst-sharded):
- Host: add self loops, sort edges by dst, shard dst nodes into 8 blocks of
  2500, pack each destination node's edges into contiguous slots of 128-slot
  chunks (<=16 dst nodes per chunk for layer 1, <=32 for layer 2). Per-edge
  source features are gathered host-side ("all-to-all the gathered source
  features") into per-slot fp16 tiles; attention a-values are likewise
  expanded per slot. Weight reparameterisations: vsrc/vdst = att @ W folds so
  attention logits come from emb directly; W2.T @ att2 folds the layer-2
  attention projections.
- Launch A (device): node/col encoders -> emb1^T shard + a1^T shard per core.
- Launch B (device): layer-1 attention (leaky+exp+softmax via masked
  numerators and a ones-matmul for denominators), aggregation as one
  128x128x128 fp16 matmul per chunk, PE transpose, W1 apply + bias + relu,
  xp2 = emb2 @ W2.T and a2 = emb2 @ w2v contractions.
- Launch C (device): layer-2 attention + aggregation (+b2, relu) + final
  linear -> logits^T slots. Host unpacks slots -> logits [10000, 128].
"""

import sys

for _p in ("/opt/trn_rl_repo", "/root/.axon_site"):
    if _p not in sys.path:
        sys.path.insert(0, _p)

import numpy as np

import concourse.bacc as bacc
import concourse.bass as bass
import concourse.tile as tile
from concourse import mybir
from concourse.bass_utils import run_bass_kernel_spmd

F32 = mybir.dt.float32
F16 = mybir.dt.float16

N_CONS = 10000
N_COLS = 10000
N = N_CONS + N_COLS
N_CORES = 8
SHARD = N // N_CORES
NEG = 0.2
GB = 16            # chunks per compute batch
WB = 8             # chunks per W1 batch (launch B)
ENC_COLS = 2560    # padded shard width for launch A (5 x 512)

_programs = {}


# ----------------------------------------------------------------------------
# host-side edge preprocessing
# ----------------------------------------------------------------------------

def _pack_edges(src, dst, lo, hi, max_nodes):
    """Pack edges with dst in [lo, hi) into 128-slot chunks.

    Each dst node's edges occupy contiguous slots within a single chunk; at
    most max_nodes nodes per chunk.
    """
    sel = (dst >= lo) & (dst < hi)
    s = src[sel]
    d = dst[sel]
    order = np.argsort(d, kind="stable")
    s = s[order]
    d = d[order]
    nodes, counts = np.unique(d, return_counts=True)
    assert counts.max() <= 128, f"degree {counts.max()} > 128 unsupported"
    offs = np.concatenate([[0], np.cumsum(counts)])

    chunk_src = []
    chunk_nodes = []
    cur_src, cur_nodes, cur_slots = [], [], 0
    for i in range(len(nodes)):
        k = int(counts[i])
        if cur_slots + k > 128 or len(cur_nodes) >= max_nodes:
            chunk_src.append(cur_src)
            chunk_nodes.append(cur_nodes)
            cur_src, cur_nodes, cur_slots = [], [], 0
        cur_src.append(s[offs[i]:offs[i + 1]])
        cur_nodes.append((int(nodes[i]), k))
        cur_slots += k
    if cur_nodes:
        chunk_src.append(cur_src)
        chunk_nodes.append(cur_nodes)

    nc_ = len(chunk_nodes)
    src_idx = np.zeros(128 * nc_, np.int64)
    dst_idx = np.zeros(128 * nc_, np.int64)
    node_col = np.full(128 * nc_, -1, np.int32)
    node_map = np.full(nc_ * max_nodes, -1, np.int32)
    for c in range(nc_):
        slot = 0
        for j, (nd, k) in enumerate(chunk_nodes[c]):
            sl = slice(128 * c + slot, 128 * c + slot + k)
            src_idx[sl] = chunk_src[c][j]
            dst_idx[sl] = nd
            node_col[sl] = j
            node_map[c * max_nodes + j] = nd
            slot += k
    return dict(n_chunks=nc_, src_idx=src_idx, dst_idx=dst_idx,
                node_col=node_col, node_map=node_map, max_nodes=max_nodes)


def _pad_chunks(pk, n_chunks_to):
    nc_, mx = pk["n_chunks"], pk["max_nodes"]
    pad = n_chunks_to - nc_
    assert pad >= 0
    if pad:
        z = np.zeros(128 * pad, np.int64)
        pk["src_idx"] = np.concatenate([pk["src_idx"], z])
        pk["dst_idx"] = np.concatenate([pk["dst_idx"], z])
        pk["node_col"] = np.concatenate(
            [pk["node_col"], np.full(128 * pad, -1, np.int32)])
        pk["node_map"] = np.concatenate(
            [pk["node_map"], np.full(mx * pad, -1, np.int32)])
    pk["n_chunks"] = n_chunks_to
    return pk


def _expand_slots(pk, table, dtype):
    """Per-slot rows table[src_idx] laid out [128, nc * width]."""
    nc_ = pk["n_chunks"]
    w = table.shape[1]
    t = table[pk["src_idx"]].reshape(nc_, 128, w).transpose(1, 0, 2)
    return np.ascontiguousarray(t.reshape(128, nc_ * w), dtype)


def _mask01(pk, dtype):
    """indicator mask [128, nc*max_nodes]: 1.0 at the slot's node col."""
    nc_, mx = pk["n_chunks"], pk["max_nodes"]
    ncol = pk["node_col"].reshape(nc_, 128)
    cols = np.arange(mx)
    m = (ncol[:, :, None] == cols[None, None, :]).astype(np.float32)
    out = m.transpose(1, 0, 2).reshape(128, nc_ * mx)
    return np.ascontiguousarray(out, dtype)


def _leaky_np(x):
    return np.where(x > 0, x, NEG * x).astype(np.float32)


# ----------------------------------------------------------------------------
# launch A: encoders
# ----------------------------------------------------------------------------

def _build_launch_a():
    nc = bacc.Bacc("TRN2", target_bir_lowering=False, debug=False)
    encT = nc.dram_tensor("encT", [16, ENC_COLS], F16, kind="ExternalInput").ap()
    encWT = nc.dram_tensor("encWT", [16, 128], F16, kind="ExternalInput").ap()
    encb = nc.dram_tensor("encb", [128, 1], F32, kind="ExternalInput").ap()
    vsV = nc.dram_tensor("vsV", [128, 16], F16, kind="ExternalInput").ap()
    embo = nc.dram_tensor("embo", [128, ENC_COLS], F32, kind="ExternalOutput").ap()
    a1o = nc.dram_tensor("a1o", [16, ENC_COLS], F32, kind="ExternalOutput").ap()

    with tile.TileContext(nc) as tc:
        with (
            tc.tile_pool(name="singles", bufs=1) as singles,
            tc.tile_pool(name="ps1", bufs=2, space="PSUM") as ps1,
            tc.tile_pool(name="ps2", bufs=2, space="PSUM") as ps2,
        ):
            encT_sb = singles.tile([16, ENC_COLS], F16)
            nc.sync.dma_start(out=encT_sb, in_=encT)
            encWT_sb = singles.tile([16, 128], F16)
            nc.sync.dma_start(out=encWT_sb, in_=encWT)
            encb_sb = singles.tile([128, 1], F32)
            nc.sync.dma_start(out=encb_sb, in_=encb)
            vsV_sb = singles.tile([128, 16], F16)
            nc.sync.dma_start(out=vsV_sb, in_=vsV)
            emb_sb = singles.tile([128, ENC_COLS], F16)
            embf_sb = singles.tile([128, ENC_COLS], F32)
            a1_sb = singles.tile([16, ENC_COLS], F32)

            nw = ENC_COLS // 512
            for w in range(nw):
                sl = slice(512 * w, 512 * (w + 1))
                p1 = ps1.tile([128, 512], F32)
                nc.tensor.matmul(out=p1, lhsT=encWT_sb, rhs=encT_sb[:, sl],
                                 start=True, stop=True)
                nc.scalar.activation(emb_sb[:, sl], p1,
                                     mybir.ActivationFunctionType.Relu,
                                     bias=encb_sb[:, 0:1])
                nc.vector.tensor_copy(embf_sb[:, sl], emb_sb[:, sl])
            for w in range(nw):
                sl = slice(512 * w, 512 * (w + 1))
                p2 = ps2.tile([16, 512], F32)
                nc.tensor.matmul(out=p2, lhsT=vsV_sb, rhs=emb_sb[:, sl],
                                 start=True, stop=True)
                nc.vector.tensor_copy(a1_sb[:, sl], p2)
            nc.sync.dma_start(out=embo, in_=embf_sb)
            nc.sync.dma_start(out=a1o, in_=a1_sb)
    nc.compile()
    return nc


# ----------------------------------------------------------------------------
# launch B: GAT layer 1 (+ W1, relu, xp2, a2)
# ----------------------------------------------------------------------------

def _build_launch_b(nchunks):
    assert nchunks % GB == 0
    nsn = nchunks * 16
    nwb = nchunks // WB

    nc = bacc.Bacc("TRN2", target_bir_lowering=False, debug=False)
    t_gx = nc.dram_tensor("gx", [128, nchunks * 132], F16,
                          kind="ExternalInput").ap()
    t_ap = nc.dram_tensor("apn", [128, nchunks * 16], F32,
                          kind="ExternalInput").ap()
    t_mask = nc.dram_tensor("mask01", [128, nchunks * 16], F16,
                            kind="ExternalInput").ap()
    t_mxr = nc.dram_tensor("mxr", [128, 8], F32, kind="ExternalInput").ap()
    t_w1t = nc.dram_tensor("w1t", [128, 8, 128], F16, kind="ExternalInput").ap()
    t_w2tv = nc.dram_tensor("w2tv", [128, 8, 132], F16,
                            kind="ExternalInput").ap()
    t_b1c = nc.dram_tensor("b1c", [128, 8], F32, kind="ExternalInput").ap()
    t_id = nc.dram_tensor("ident", [128, 128], F16, kind="ExternalInput").ap()
    t_xp2o = nc.dram_tensor("xp2o", [nsn, 130], F32, kind="ExternalOutput").ap()

    with tile.TileContext(nc) as tc:
        with (
            tc.tile_pool(name="singles", bufs=1) as singles,
            tc.tile_pool(name="gt", bufs=2) as gt,
            tc.tile_pool(name="at", bufs=2) as at,
            tc.tile_pool(name="mt", bufs=2) as mt,
            tc.tile_pool(name="et", bufs=2) as et,
            tc.tile_pool(name="pt", bufs=2) as pt,
            tc.tile_pool(name="asb", bufs=2) as asb,
            tc.tile_pool(name="rr", bufs=4) as rr,
            tc.tile_pool(name="atb", bufs=2) as atb,
            tc.tile_pool(name="e2t", bufs=2) as e2tp,
            tc.tile_pool(name="xsb", bufs=2) as xsb,
            tc.tile_pool(name="aggps", bufs=2, space="PSUM") as aggps,
            tc.tile_pool(name="atps", bufs=1, space="PSUM") as atps,
            tc.tile_pool(name="o1ps", bufs=1, space="PSUM") as o1ps,
            tc.tile_pool(name="x2ps", bufs=1, space="PSUM") as x2ps,
        ):
            w1t_sb = singles.tile([128, 8, 128], F16)
            nc.sync.dma_start(out=w1t_sb, in_=t_w1t)
            w2tv_sb = singles.tile([128, 8, 132], F16)
            nc.sync.dma_start(out=w2tv_sb, in_=t_w2tv)
            b1c_sb = singles.tile([128, 8], F32)
            nc.sync.dma_start(out=b1c_sb, in_=t_b1c)
            id_sb = singles.tile([128, 128], F16)
            nc.sync.dma_start(out=id_sb, in_=t_id)
            mxr_sb = singles.tile([128, 8], F32)
            nc.sync.dma_start(out=mxr_sb, in_=t_mxr)

            ngb = nchunks // GB
            gtiles = [None] * ngb
            ptiles = [None] * ngb

            def issue_batch(gb):
                gsl = slice(gb * GB * 132, (gb + 1) * GB * 132)
                g = gt.tile([128, GB, 132], F16, tag="g")
                nc.sync.dma_start(out=g, in_=t_gx[:, gsl])
                ap_ = at.tile([128, GB, 16], F32, tag="ap")
                asl = slice(gb * GB * 16, (gb + 1) * GB * 16)
                nc.sync.dma_start(out=ap_, in_=t_ap[:, asl])
                mask = mt.tile([128, GB, 16], F16, tag="mask")
                msl = slice(gb * GB * 16, (gb + 1) * GB * 16)
                nc.sync.dma_start(out=mask, in_=t_mask[:, msl])
                # e = leaky(asrc + adst); p = exp(e) * maskexp
                st = et.tile([128, GB, 8], F32, tag="st")
                nc.vector.tensor_tensor(out=st, in0=ap_[:, :, 0:8],
                                        in1=ap_[:, :, 8:16],
                                        op=mybir.AluOpType.add)
                lk = et.tile([128, GB, 8], F32, tag="lk")
                nc.vector.tensor_scalar_mul(lk, st, NEG)
                ee = et.tile([128, GB, 8], F32, tag="ee")
                nc.vector.tensor_tensor(out=ee, in0=st, in1=lk,
                                        op=mybir.AluOpType.max)
                ee2 = et.tile([128, GB, 8], F32, tag="ee2")
                mx_rep = bass.AP(tensor=mxr_sb.tensor, offset=mxr_sb.offset,
                                 ap=[mxr_sb.ap[0], [0, GB], mxr_sb.ap[1]])
                nc.vector.tensor_tensor(out=ee2, in0=ee, in1=mx_rep,
                                        op=mybir.AluOpType.subtract)
                ex = et.tile([128, GB, 8], F16, tag="ex")
                nc.scalar.activation(ex, ee2, mybir.ActivationFunctionType.Exp)
                p = pt.tile([128, GB, 16, 8], F16, tag="p")
                ex_rep = bass.AP(tensor=ex.tensor, offset=ex.offset,
                                 ap=[ex.ap[0], ex.ap[1], [0, 16], ex.ap[2]])
                mask_rep = bass.AP(tensor=mask.tensor, offset=mask.offset,
                                   ap=[mask.ap[0], mask.ap[1], mask.ap[2],
                                       [0, 8]])
                nc.vector.tensor_tensor(out=p, in0=ex_rep, in1=mask_rep,
                                        op=mybir.AluOpType.mult)
                return g, p

            for wb in range(nwb):
                if wb % 2 == 0:
                    gtiles[wb // 2], ptiles[wb // 2] = issue_batch(wb // 2)
                g, p = gtiles[wb // 2], ptiles[wb // 2]
                atb_t = atb.tile([128, WB, 128], F16, tag="atb")
                for half in range(2):
                    aggf = aggps.tile([128, 4, 256], F32, tag="agg")
                    for q in range(4):
                        cb = (wb % 2) * WB + half * 4 + q
                        p_c = p[:, cb, :, :].rearrange("p a b -> p (a b)")
                        nc.tensor.matmul(out=aggf[:, q, 0:129], lhsT=p_c,
                                         rhs=g[:, cb, 0:129],
                                         start=True, stop=True)
                    rc4 = rr.tile([128, 4], F32, tag="rc")
                    nc.vector.reciprocal(rc4, aggf[:, :, 128:129])
                    a4 = asb.tile([128, 4, 128], F16, tag="a")
                    rc4_rep = bass.AP(tensor=rc4.tensor, offset=rc4.offset,
                                      ap=[rc4.ap[0], rc4.ap[1], [0, 128]])
                    nc.vector.tensor_tensor(out=a4, in0=aggf[:, :, 0:128],
                                            in1=rc4_rep,
                                            op=mybir.AluOpType.mult)
                    atpf = atps.tile([128, 4, 128], F16, tag="atp")
                    for q in range(4):
                        nc.tensor.transpose(out=atpf[:, q, :], in_=a4[:, q, :],
                                            identity=id_sb)
                    nc.scalar.activation(
                        atb_t[:, half * 4:(half + 1) * 4, :], atpf,
                        mybir.ActivationFunctionType.Copy)
                # W1 apply + bias + relu -> emb2T; then xp2/a2 contraction
                o1 = o1ps.tile([128, 8, 128], F32, tag="o1")
                atb_r = atb_t.rearrange("p c (n h) -> p h c n", h=8)
                for h in range(8):
                    nc.tensor.matmul(
                        out=o1[:, h, :],
                        lhsT=w1t_sb[:, h, :],
                        rhs=atb_r[:, h, :, :],
                        start=True, stop=True)
                t1 = e2tp.tile([128, 8, 128], F32, tag="t1")
                b1_rep = bass.AP(tensor=b1c_sb.tensor, offset=b1c_sb.offset,
                                 ap=[b1c_sb.ap[0], b1c_sb.ap[1], [0, 128]])
                nc.vector.tensor_tensor(out=t1, in0=o1, in1=b1_rep,
                                        op=mybir.AluOpType.add)
                e2 = e2tp.tile([128, 8, 128], F16, tag="e2")
                nc.vector.tensor_scalar_max(e2, t1, 0.0)
                x2 = x2ps.tile([128, 132], F32, tag="x2")
                for h in range(8):
                    nc.tensor.matmul(out=x2[:, 0:130], lhsT=e2[:, h, :],
                                     rhs=w2tv_sb[:, h, 0:130],
                                     start=(h == 0), stop=(h == 7))
                x2_sb = xsb.tile([128, 130], F32, tag="x2sb")
                nc.scalar.activation(x2_sb, x2[:, 0:130],
                                     mybir.ActivationFunctionType.Copy)
                nc.sync.dma_start(out=t_xp2o[wb * 128:(wb + 1) * 128, :],
                                  in_=x2_sb)
    nc.compile()
    return nc


# ----------------------------------------------------------------------------
# launch C: GAT layer 2 + final linear
# ----------------------------------------------------------------------------

def _build_launch_c(nchunks):
    assert nchunks % GB == 0
    nsn = nchunks * 32

    nc = bacc.Bacc("TRN2", target_bir_lowering=False, debug=False)
    t_gx = nc.dram_tensor("gx2", [128, nchunks * 132], F16,
                          kind="ExternalInput").ap()
    t_ap = nc.dram_tensor("apn2", [128, nchunks * 2], F32,
                          kind="ExternalInput").ap()
    t_mask = nc.dram_tensor("mask012", [128, nchunks * 32], F16,
                            kind="ExternalInput").ap()
    t_mx2 = nc.dram_tensor("mx2c", [128, 1], F32, kind="ExternalInput").ap()
    t_oWT = nc.dram_tensor("outWT", [128, 128], F16, kind="ExternalInput").ap()
    t_ob = nc.dram_tensor("outb", [128, 1], F32, kind="ExternalInput").ap()
    t_b2 = nc.dram_tensor("b2c", [128, 1], F32, kind="ExternalInput").ap()
    t_id = nc.dram_tensor("ident2", [128, 128], F16, kind="ExternalInput").ap()
    t_lgo = nc.dram_tensor("lgo", [128, nsn], F32, kind="ExternalOutput").ap()

    with tile.TileContext(nc) as tc:
        with (
            tc.tile_pool(name="singles", bufs=1) as singles,
            tc.tile_pool(name="gt", bufs=2) as gt,
            tc.tile_pool(name="at", bufs=2) as at,
            tc.tile_pool(name="mt", bufs=2) as mt,
            tc.tile_pool(name="et", bufs=2) as et,
            tc.tile_pool(name="pt", bufs=2) as pt,
            tc.tile_pool(name="asb", bufs=2) as asb,
            tc.tile_pool(name="rr", bufs=4) as rr,
            tc.tile_pool(name="lg", bufs=2) as lgp,
            tc.tile_pool(name="aggps", bufs=4, space="PSUM") as aggps,
            tc.tile_pool(name="atps", bufs=2, space="PSUM") as atps,
            tc.tile_pool(name="lgps", bufs=2, space="PSUM") as lgps,
        ):
            oWT_sb = singles.tile([128, 128], F16)
            nc.sync.dma_start(out=oWT_sb, in_=t_oWT)
            ob_sb = singles.tile([128, 1], F32)
            nc.sync.dma_start(out=ob_sb, in_=t_ob)
            b2_sb = singles.tile([128, 1], F32)
            nc.sync.dma_start(out=b2_sb, in_=t_b2)
            id_sb = singles.tile([128, 128], F16)
            nc.sync.dma_start(out=id_sb, in_=t_id)
            mx2_sb = singles.tile([128, 1], F32)
            nc.sync.dma_start(out=mx2_sb, in_=t_mx2)
            e3t_sb = singles.tile([128, nsn], F16)

            ngb = nchunks // GB
            for gb in range(ngb):
                gsl = slice(gb * GB * 132, (gb + 1) * GB * 132)
                g = gt.tile([128, GB, 132], F16, tag="g")
                nc.sync.dma_start(out=g, in_=t_gx[:, gsl])
                ap_ = at.tile([128, GB, 2], F32, tag="ap")
                asl = slice(gb * GB * 2, (gb + 1) * GB * 2)
                nc.sync.dma_start(out=ap_, in_=t_ap[:, asl])
                mask = mt.tile([128, GB, 32], F16, tag="mask")
                msl = slice(gb * GB * 32, (gb + 1) * GB * 32)
                nc.sync.dma_start(out=mask, in_=t_mask[:, msl])
                st = et.tile([128, GB], F32, tag="st")
                nc.vector.tensor_tensor(out=st, in0=ap_[:, :, 0],
                                        in1=ap_[:, :, 1],
                                        op=mybir.AluOpType.add)
                lk = et.tile([128, GB], F32, tag="lk")
                nc.vector.tensor_scalar_mul(lk, st, NEG)
                ee = et.tile([128, GB], F32, tag="ee")
                nc.vector.tensor_tensor(out=ee, in0=st, in1=lk,
                                        op=mybir.AluOpType.max)
                ee2 = et.tile([128, GB], F32, tag="ee2")
                nc.vector.tensor_scalar_sub(ee2, ee, mx2_sb[:, 0:1])
                ex = et.tile([128, GB], F16, tag="ex")
                nc.scalar.activation(ex, ee2, mybir.ActivationFunctionType.Exp)
                p = pt.tile([128, GB, 32], F16, tag="p")
                ex_rep = bass.AP(tensor=ex.tensor, offset=ex.offset,
                                 ap=[ex.ap[0], ex.ap[1], [0, 32]])
                nc.vector.tensor_tensor(out=p, in0=ex_rep, in1=mask,
                                        op=mybir.AluOpType.mult)

                for grp in range(GB // 4):
                    aggc = aggps.tile([128, 132], F32, tag="agg")
                    for q in range(4):
                        cb = grp * 4 + q
                        nc.tensor.matmul(out=aggc[32 * q:32 * (q + 1), 0:129],
                                         lhsT=p[:, cb, :],
                                         rhs=g[:, cb, 0:129],
                                         start=True, stop=True,
                                         tile_position=(0, 32 * q))
                    rc = rr.tile([128, 1], F32, tag="rc")
                    nc.vector.reciprocal(rc, aggc[:, 128:129])
                    a4 = asb.tile([128, 128], F16, tag="a")
                    nc.scalar.activation(a4, aggc[:, 0:128],
                                         mybir.ActivationFunctionType.Copy,
                                         scale=rc[:, 0:1])
                    atp = atps.tile([128, 128], F16, tag="atp")
                    nc.tensor.transpose(out=atp, in_=a4, identity=id_sb)
                    c0 = (gb * 4 + grp) * 128
                    nc.scalar.activation(e3t_sb[:, c0:c0 + 128], atp,
                                         mybir.ActivationFunctionType.Relu,
                                         bias=b2_sb[:, 0:1])
            # logits^T = outW.T.T @ emb3T + out_b
            nwin = nsn // 512
            for w in range(nwin):
                sl = slice(512 * w, 512 * (w + 1))
                lp = lgps.tile([128, 512], F32, tag="lg")
                nc.tensor.matmul(out=lp, lhsT=oWT_sb, rhs=e3t_sb[:, sl],
                                 start=True, stop=True)
                lsb = lgp.tile([128, 512], F32, tag="lsb")
                nc.vector.tensor_scalar_add(lsb, lp, ob_sb[:, 0:1])
                nc.sync.dma_start(out=t_lgo[:, sl], in_=lsb)
    nc.compile()
    return nc


# ----------------------------------------------------------------------------
# main entry
# ----------------------------------------------------------------------------

def kernel(**inputs):
    cs = np.ascontiguousarray(inputs["constraints_state"], np.float32)
    xs = np.ascontiguousarray(inputs["columns_state"], np.float32)
    node_W = np.asarray(inputs["node_W"], np.float32)
    node_b = np.asarray(inputs["node_b"], np.float32)
    col_W = np.asarray(inputs["col_W"], np.float32)
    col_b = np.asarray(inputs["col_b"], np.float32)
    W1 = np.asarray(inputs["W1"], np.float32)
    att_src1 = np.asarray(inputs["att_src1"], np.float32)
    att_dst1 = np.asarray(inputs["att_dst1"], np.float32)
    b1 = np.asarray(inputs["b1"], np.float32)
    W2 = np.asarray(inputs["W2"], np.float32)
    att_src2 = np.asarray(inputs["att_src2"], np.float32)
    att_dst2 = np.asarray(inputs["att_dst2"], np.float32)
    b2 = np.asarray(inputs["b2"], np.float32)
    out_W = np.asarray(inputs["out_W"], np.float32)
    out_b = np.asarray(inputs["out_b"], np.float32)
    edges = np.asarray(inputs["edges"]).astype(np.int64)

    # ---- weight folds
    W1h = W1.reshape(8, 128, 128)
    vsrc1 = np.einsum("hc,hcd->hd", att_src1, W1h).astype(np.float32)
    vdst1 = np.einsum("hc,hcd->hd", att_dst1, W1h).astype(np.float32)
    w2v = (W2.T @ np.stack([att_src2[0], att_dst2[0]], 1)).astype(np.float32)

    # ---- edges + self loops, per-core packing
    loops = np.arange(N, dtype=np.int64)
    src = np.concatenate([edges[0], loops])
    dst = np.concatenate([edges[1], loops])
    packs1, packs2 = [], []
    for core in range(N_CORES):
        lo, hi = core * SHARD, (core + 1) * SHARD
        packs1.append(_pack_edges(src, dst, lo, hi, 16))
        packs2.append(_pack_edges(src, dst, lo, hi, 32))

    def _roundup(x, m):
        return (x + m - 1) // m * m

    nc1 = _roundup(max(p["n_chunks"] for p in packs1), GB)
    nc2 = _roundup(max(p["n_chunks"] for p in packs2), GB)
    packs1 = [_pad_chunks(p, nc1) for p in packs1]
    packs2 = [_pad_chunks(p, nc2) for p in packs2]

    # ---- compile programs (cached)
    if "a" not in _programs:
        _programs["a"] = _build_launch_a()
    if ("b", nc1) not in _programs:
        _programs[("b", nc1)] = _build_launch_b(nc1)
    if ("c", nc2) not in _programs:
        _programs[("c", nc2)] = _build_launch_c(nc2)
    prog_a = _programs["a"]
    prog_b = _programs[("b", nc1)]
    prog_c = _programs[("c", nc2)]

    # ---- launch A
    vsV = np.concatenate([vsrc1.T, vdst1.T], 1).astype(np.float32)
    in_a = []
    for core in range(N_CORES):
        lo = core * SHARD
        if lo < N_CONS:
            feat = np.tile(cs[lo:lo + SHARD], (1, 2))
            encW = np.concatenate([node_W, np.zeros((128, 8), np.float32)], 1)
            encb_ = node_b
        else:
            feat = np.tile(xs[lo - N_CONS:lo - N_CONS + SHARD], (1, 2))
            encW = col_W
            encb_ = col_b
        encT = np.zeros((16, ENC_COLS), np.float32)
        encT[:feat.shape[1], :SHARD] = feat.T
        in_a.append({
            "encT": encT.astype(np.float16),
            "encWT": np.ascontiguousarray(encW.T, np.float16),
            "encb": encb_.reshape(128, 1).astype(np.float32),
            "vsV": vsV.astype(np.float16),
        })
    res_a = _run(prog_a, in_a, "A")
    emb1 = np.concatenate(
        [res_a.results[c]["embo"][:, :SHARD].T for c in range(N_CORES)], 0)
    a1 = np.concatenate(
        [res_a.results[c]["a1o"][:, :SHARD].T for c in range(N_CORES)], 0)
    a1 = np.ascontiguousarray(a1, np.float32)               # [N, 16]

    # ---- host: expanded per-slot inputs for launch B
    emb1e = np.zeros((N, 132), np.float16)
    emb1e[:, 0:128] = emb1.astype(np.float16)
    emb1e[:, 128] = 1.0
    mx1 = _leaky_np(a1[:, 0:8].max(0) + a1[:, 8:16].max(0))

    ident = np.eye(128, dtype=np.float16)
    w1t = np.ascontiguousarray(W1h.transpose(2, 0, 1), np.float16)
    w2tv = np.zeros((128, 8, 132), np.float16)
    w2tv[:, :, 0:128] = W2.T.reshape(8, 128, 128).transpose(1, 0, 2)
    w2tv[:, :, 128:130] = w2v.reshape(8, 128, 2).transpose(1, 0, 2)
    b1c = np.ascontiguousarray(b1.reshape(8, 128).T, np.float32)

    in_b = []
    for core in range(N_CORES):
        pk = packs1[core]
        nc_ = pk["n_chunks"]
        apn = np.concatenate([
            a1[pk["src_idx"], 0:8], a1[pk["dst_idx"], 8:16]], 1)
        apn = np.ascontiguousarray(
            apn.reshape(nc_, 128, 16).transpose(1, 0, 2).reshape(128, -1),
            np.float32)
        in_b.append({
            "gx": _expand_slots(pk, emb1e, np.float16),
            "apn": apn,
            "mask01": _mask01(pk, np.float16),
            "mxr": np.tile(mx1, (128, 1)).astype(np.float32),
            "w1t": w1t, "w2tv": w2tv, "b1c": b1c, "ident": ident,
        })
    res_b = _run(prog_b, in_b, "B")

    # ---- host: assemble xp2 / a2 tables
    tab2e = np.zeros((N, 132), np.float16)
    tab2e[:, 128] = 1.0
    a2 = np.zeros((N, 2), np.float32)
    for core in range(N_CORES):
        nm = packs1[core]["node_map"]
        valid = nm >= 0
        xo = res_b.results[core]["xp2o"]
        tab2e[nm[valid], 0:128] = xo[valid, 0:128].astype(np.float16)
        a2[nm[valid]] = xo[valid, 128:130]
    mx2 = _leaky_np(np.array(
        [a2[:, 0].max() + a2[:, 1].max()], np.float32))

    in_c = []
    for core in range(N_CORES):
        pk = packs2[core]
        nc_ = pk["n_chunks"]
        apn2 = np.stack([a2[pk["src_idx"], 0], a2[pk["dst_idx"], 1]], 1)
        apn2 = np.ascontiguousarray(
            apn2.reshape(nc_, 128, 2).transpose(1, 0, 2).reshape(128, -1),
            np.float32)
        in_c.append({
            "gx2": _expand_slots(pk, tab2e, np.float16),
            "apn2": apn2,
            "mask012": _mask01(pk, np.float16),
            "mx2c": np.full((128, 1), mx2[0], np.float32),
            "outWT": np.ascontiguousarray(out_W.T, np.float16),
            "outb": out_b.reshape(128, 1).astype(np.float32),
            "b2c": b2.reshape(128, 1).astype(np.float32),
            "ident2": ident,
        })
    res_c = _run(prog_c, in_c, "C")

    logits = np.zeros((N, 128), np.float32)
    for core in range(N_CORES):
        nm = packs2[core]["node_map"]
        valid = nm >= 0
        logits[nm[valid]] = res_c.results[core]["lgo"][:, valid].T

    return logits[-N_COLS:].astype(np.float32)


_trace = {"enable": False, "dir": None, "exec_ns": {}}


def _run(prog, in_maps, tag):
    kwargs = {}
    if _trace["enable"]:
        import os
        d = os.path.join(_trace["dir"], tag)
        os.makedirs(d, exist_ok=True)
        kwargs = dict(trace=True, tmpdir=d)
    res = run_bass_kernel_spmd(prog, in_maps, core_ids=list(range(N_CORES)),
                               **kwargs)
    _trace["exec_ns"][tag] = res.exec_time_ns
    return res


# revision 18
# speedup vs baseline: 1.1574x; 1.0796x over previous
"""Trainium2 Bass kernel for nn_GAT_66821101191795 (2-layer GAT, 8 NeuronCores).

Strategy (graph/data parallel, dst-sharded):
- Host: add self loops, sort edges by dst, shard dst nodes into 8 blocks of
  2500, pack each destination node's edges into contiguous slots of 128-slot
  chunks (<=16 dst nodes per chunk for layer 1, <=32 for layer 2). Per-edge
  source features are gathered host-side ("all-to-all the gathered source
  features") into per-slot fp16 tiles; attention a-values are likewise
  expanded per slot. Weight reparameterisations: vsrc/vdst = att @ W folds so
  attention logits come from emb directly; W2.T @ att2 folds the layer-2
  attention projections.
- Launch A (device): node/col encoders -> emb1^T shard + a1^T shard per core.
- Launch B (device): layer-1 attention (leaky+exp+softmax via masked
  numerators and a ones-matmul for denominators), aggregation as one
  128x128x128 fp16 matmul per chunk, PE transpose, W1 apply + bias + relu,
  xp2 = emb2 @ W2.T and a2 = emb2 @ w2v contractions.
- Launch C (device): layer-2 attention + aggregation (+b2, relu) + final
  linear -> logits^T slots. Host unpacks slots -> logits [10000, 128].
"""

import sys

for _p in ("/opt/trn_rl_repo", "/root/.axon_site"):
    if _p not in sys.path:
        sys.path.insert(0, _p)

import numpy as np

import concourse.bacc as bacc
import concourse.bass as bass
import concourse.tile as tile
from concourse import mybir
from concourse.bass_utils import run_bass_kernel_spmd

F32 = mybir.dt.float32
F16 = mybir.dt.float16

N_CONS = 10000
N_COLS = 10000
N = N_CONS + N_COLS
N_CORES = 8
SHARD = N // N_CORES
NEG = 0.2
GB = 16            # chunks per compute batch
WB = 8             # chunks per W1 batch (launch B)
ENC_COLS = 2560    # padded shard width for launch A (5 x 512)

_programs = {}


# ----------------------------------------------------------------------------
# host-side edge preprocessing
# ----------------------------------------------------------------------------

def _pack_edges(src, dst, lo, hi, max_nodes):
    """Pack edges with dst in [lo, hi) into 128-slot chunks.

    Each dst node's edges occupy contiguous slots within a single chunk; at
    most max_nodes nodes per chunk.
    """
    sel = (dst >= lo) & (dst < hi)
    s = src[sel]
    d = dst[sel]
    order = np.argsort(d, kind="stable")
    s = s[order]
    d = d[order]
    nodes, counts = np.unique(d, return_counts=True)
    assert counts.max() <= 128, f"degree {counts.max()} > 128 unsupported"
    offs = np.concatenate([[0], np.cumsum(counts)])

    chunk_src = []
    chunk_nodes = []
    cur_src, cur_nodes, cur_slots = [], [], 0
    for i in range(len(nodes)):
        k = int(counts[i])
        if cur_slots + k > 128 or len(cur_nodes) >= max_nodes:
            chunk_src.append(cur_src)
            chunk_nodes.append(cur_nodes)
            cur_src, cur_nodes, cur_slots = [], [], 0
        cur_src.append(s[offs[i]:offs[i + 1]])
        cur_nodes.append((int(nodes[i]), k))
        cur_slots += k
    if cur_nodes:
        chunk_src.append(cur_src)
        chunk_nodes.append(cur_nodes)

    nc_ = len(chunk_nodes)
    src_idx = np.zeros(128 * nc_, np.int64)
    dst_idx = np.zeros(128 * nc_, np.int64)
    node_col = np.full(128 * nc_, -1, np.int32)
    node_map = np.full(nc_ * max_nodes, -1, np.int32)
    for c in range(nc_):
        slot = 0
        for j, (nd, k) in enumerate(chunk_nodes[c]):
            sl = slice(128 * c + slot, 128 * c + slot + k)
            src_idx[sl] = chunk_src[c][j]
            dst_idx[sl] = nd
            node_col[sl] = j
            node_map[c * max_nodes + j] = nd
            slot += k
    return dict(n_chunks=nc_, src_idx=src_idx, dst_idx=dst_idx,
                node_col=node_col, node_map=node_map, max_nodes=max_nodes)


def _pad_chunks(pk, n_chunks_to):
    nc_, mx = pk["n_chunks"], pk["max_nodes"]
    pad = n_chunks_to - nc_
    assert pad >= 0
    if pad:
        z = np.zeros(128 * pad, np.int64)
        pk["src_idx"] = np.concatenate([pk["src_idx"], z])
        pk["dst_idx"] = np.concatenate([pk["dst_idx"], z])
        pk["node_col"] = np.concatenate(
            [pk["node_col"], np.full(128 * pad, -1, np.int32)])
        pk["node_map"] = np.concatenate(
            [pk["node_map"], np.full(mx * pad, -1, np.int32)])
    pk["n_chunks"] = n_chunks_to
    return pk


def _expand_slots(pk, table, dtype):
    """Per-slot rows table[src_idx] laid out [128, nc * width]."""
    nc_ = pk["n_chunks"]
    w = table.shape[1]
    t = table[pk["src_idx"]].reshape(nc_, 128, w).transpose(1, 0, 2)
    return np.ascontiguousarray(t.reshape(128, nc_ * w), dtype)


def _mask01(pk, dtype):
    """indicator mask [128, nc*max_nodes]: 1.0 at the slot's node col."""
    nc_, mx = pk["n_chunks"], pk["max_nodes"]
    ncol = pk["node_col"].reshape(nc_, 128)
    cols = np.arange(mx)
    m = (ncol[:, :, None] == cols[None, None, :]).astype(np.float32)
    out = m.transpose(1, 0, 2).reshape(128, nc_ * mx)
    return np.ascontiguousarray(out, dtype)


def _leaky_np(x):
    return np.where(x > 0, x, NEG * x).astype(np.float32)


# ----------------------------------------------------------------------------
# launch A: encoders
# ----------------------------------------------------------------------------

def _build_launch_a():
    nc = bacc.Bacc("TRN2", target_bir_lowering=False, debug=False)
    encT = nc.dram_tensor("encT", [16, ENC_COLS], F16, kind="ExternalInput").ap()
    encWT = nc.dram_tensor("encWT", [16, 128], F16, kind="ExternalInput").ap()
    encb = nc.dram_tensor("encb", [128, 1], F32, kind="ExternalInput").ap()
    vsV = nc.dram_tensor("vsV", [128, 16], F16, kind="ExternalInput").ap()
    embo = nc.dram_tensor("embo", [128, ENC_COLS], F16, kind="ExternalOutput").ap()
    a1o = nc.dram_tensor("a1o", [16, ENC_COLS], F32, kind="ExternalOutput").ap()

    with tile.TileContext(nc) as tc:
        with (
            tc.tile_pool(name="singles", bufs=1) as singles,
            tc.tile_pool(name="ps1", bufs=2, space="PSUM") as ps1,
            tc.tile_pool(name="ps2", bufs=2, space="PSUM") as ps2,
        ):
            encT_sb = singles.tile([16, ENC_COLS], F16)
            nc.sync.dma_start(out=encT_sb, in_=encT)
            encWT_sb = singles.tile([16, 128], F16)
            nc.sync.dma_start(out=encWT_sb, in_=encWT)
            encb_sb = singles.tile([128, 1], F32)
            nc.sync.dma_start(out=encb_sb, in_=encb)
            vsV_sb = singles.tile([128, 16], F16)
            nc.sync.dma_start(out=vsV_sb, in_=vsV)
            emb_sb = singles.tile([128, ENC_COLS], F16)
            a1_sb = singles.tile([16, ENC_COLS], F32)

            nw = ENC_COLS // 512
            for w in range(nw):
                sl = slice(512 * w, 512 * (w + 1))
                p1 = ps1.tile([128, 512], F32)
                nc.tensor.matmul(out=p1, lhsT=encWT_sb, rhs=encT_sb[:, sl],
                                 start=True, stop=True)
                nc.scalar.activation(emb_sb[:, sl], p1,
                                     mybir.ActivationFunctionType.Relu,
                                     bias=encb_sb[:, 0:1])
            for w in range(nw):
                sl = slice(512 * w, 512 * (w + 1))
                p2 = ps2.tile([16, 512], F32)
                nc.tensor.matmul(out=p2, lhsT=vsV_sb, rhs=emb_sb[:, sl],
                                 start=True, stop=True)
                nc.vector.tensor_copy(a1_sb[:, sl], p2)
            nc.sync.dma_start(out=embo, in_=emb_sb)
            nc.sync.dma_start(out=a1o, in_=a1_sb)
    nc.compile()
    return nc


# ----------------------------------------------------------------------------
# launch B: GAT layer 1 (+ W1, relu, xp2, a2)
# ----------------------------------------------------------------------------

def _build_launch_b(nchunks, b1_zero=False):
    assert nchunks % GB == 0
    nsn = nchunks * 16
    nwb = nchunks // WB

    nc = bacc.Bacc("TRN2", target_bir_lowering=False, debug=False)
    t_gx = nc.dram_tensor("gx", [128, nchunks * 132], F16,
                          kind="ExternalInput").ap()
    t_ap = nc.dram_tensor("apn", [128, nchunks * 16], F32,
                          kind="ExternalInput").ap()
    t_mask = nc.dram_tensor("mask01", [128, nchunks * 16], F16,
                            kind="ExternalInput").ap()
    t_mxr = nc.dram_tensor("mxr", [128, 8], F32, kind="ExternalInput").ap()
    t_w1t = nc.dram_tensor("w1t", [128, 8, 128], F16, kind="ExternalInput").ap()
    t_w2tv = nc.dram_tensor("w2tv", [128, 8, 132], F16,
                            kind="ExternalInput").ap()
    t_b1c = nc.dram_tensor("b1c", [128, 8], F32, kind="ExternalInput").ap()
    t_id = nc.dram_tensor("ident", [128, 128], F16, kind="ExternalInput").ap()
    t_xp2o = nc.dram_tensor("xp2o", [nsn, 130], F32, kind="ExternalOutput").ap()

    with tile.TileContext(nc) as tc:
        with (
            tc.tile_pool(name="singles", bufs=1) as singles,
            tc.tile_pool(name="gt", bufs=3) as gt,
            tc.tile_pool(name="at", bufs=3) as at,
            tc.tile_pool(name="mt", bufs=3) as mt,
            tc.tile_pool(name="et", bufs=3) as et,
            tc.tile_pool(name="pt", bufs=3) as pt,
            tc.tile_pool(name="asb", bufs=4) as asb,
            tc.tile_pool(name="rr", bufs=8) as rr,
            tc.tile_pool(name="atb", bufs=3) as atb,
            tc.tile_pool(name="e2t", bufs=3) as e2tp,
            tc.tile_pool(name="xsb", bufs=3) as xsb,
            tc.tile_pool(name="aggps", bufs=2, space="PSUM") as aggps,
            tc.tile_pool(name="atps", bufs=1, space="PSUM") as atps,
            tc.tile_pool(name="o1ps", bufs=1, space="PSUM") as o1ps,
            tc.tile_pool(name="x2ps", bufs=1, space="PSUM") as x2ps,
        ):
            w1t_sb = singles.tile([128, 8, 128], F16)
            nc.sync.dma_start(out=w1t_sb, in_=t_w1t)
            w2tv_sb = singles.tile([128, 8, 132], F16)
            nc.sync.dma_start(out=w2tv_sb, in_=t_w2tv)
            b1c_sb = singles.tile([128, 8], F32)
            nc.sync.dma_start(out=b1c_sb, in_=t_b1c)
            id_sb = singles.tile([128, 128], F16)
            nc.sync.dma_start(out=id_sb, in_=t_id)
            mxr_sb = singles.tile([128, 8], F32)
            nc.sync.dma_start(out=mxr_sb, in_=t_mxr)

            ngb = nchunks // GB
            gtiles = [None] * ngb
            ptiles = [None] * ngb

            def issue_batch(gb):
                gsl = slice(gb * GB * 132, (gb + 1) * GB * 132)
                g = gt.tile([128, GB, 132], F16, tag="g")
                nc.sync.dma_start(out=g, in_=t_gx[:, gsl])
                ap_ = at.tile([128, GB, 16], F32, tag="ap")
                asl = slice(gb * GB * 16, (gb + 1) * GB * 16)
                nc.sync.dma_start(out=ap_, in_=t_ap[:, asl])
                mask = mt.tile([128, GB, 16], F16, tag="mask")
                msl = slice(gb * GB * 16, (gb + 1) * GB * 16)
                nc.sync.dma_start(out=mask, in_=t_mask[:, msl])
                # e = leaky(asrc + adst); p = exp(e) * maskexp
                st = et.tile([128, GB, 8], F32, tag="st")
                nc.vector.tensor_tensor(out=st, in0=ap_[:, :, 0:8],
                                        in1=ap_[:, :, 8:16],
                                        op=mybir.AluOpType.add)
                lk = et.tile([128, GB, 8], F32, tag="lk")
                nc.vector.tensor_scalar_mul(lk, st, NEG)
                ee = et.tile([128, GB, 8], F32, tag="ee")
                nc.vector.tensor_tensor(out=ee, in0=st, in1=lk,
                                        op=mybir.AluOpType.max)
                ee2 = et.tile([128, GB, 8], F32, tag="ee2")
                mx_rep = bass.AP(tensor=mxr_sb.tensor, offset=mxr_sb.offset,
                                 ap=[mxr_sb.ap[0], [0, GB], mxr_sb.ap[1]])
                nc.vector.tensor_tensor(out=ee2, in0=ee, in1=mx_rep,
                                        op=mybir.AluOpType.subtract)
                ex = et.tile([128, GB, 8], F16, tag="ex")
                nc.scalar.activation(ex, ee2, mybir.ActivationFunctionType.Exp)
                p = pt.tile([128, GB, 16, 8], F16, tag="p")
                ex_rep = bass.AP(tensor=ex.tensor, offset=ex.offset,
                                 ap=[ex.ap[0], ex.ap[1], [0, 16], ex.ap[2]])
                mask_rep = bass.AP(tensor=mask.tensor, offset=mask.offset,
                                   ap=[mask.ap[0], mask.ap[1], mask.ap[2],
                                       [0, 8]])
                nc.vector.tensor_tensor(out=p, in0=ex_rep, in1=mask_rep,
                                        op=mybir.AluOpType.mult)
                return g, p

            for wb in range(nwb):
                if wb % 2 == 0:
                    gtiles[wb // 2], ptiles[wb // 2] = issue_batch(wb // 2)
                g, p = gtiles[wb // 2], ptiles[wb // 2]
                atb_t = atb.tile([128, WB, 128], F16, tag="atb")
                for half in range(2):
                    aggf = aggps.tile([128, 4, 256], F32, tag="agg")
                    for q in range(4):
                        cb = (wb % 2) * WB + half * 4 + q
                        p_c = p[:, cb, :, :].rearrange("p a b -> p (a b)")
                        nc.tensor.matmul(out=aggf[:, q, 0:129], lhsT=p_c,
                                         rhs=g[:, cb, 0:129],
                                         start=True, stop=True)
                    rc4 = rr.tile([128, 4], F32, tag="rc")
                    nc.vector.reciprocal(rc4, aggf[:, :, 128:129])
                    a4 = asb.tile([128, 4, 128], F16, tag="a")
                    rc4_rep = bass.AP(tensor=rc4.tensor, offset=rc4.offset,
                                      ap=[rc4.ap[0], rc4.ap[1], [0, 128]])
                    nc.vector.tensor_tensor(out=a4, in0=aggf[:, :, 0:128],
                                            in1=rc4_rep,
                                            op=mybir.AluOpType.mult)
                    atpf = atps.tile([128, 4, 128], F16, tag="atp")
                    for q in range(4):
                        nc.tensor.transpose(out=atpf[:, q, :], in_=a4[:, q, :],
                                            identity=id_sb)
                    nc.scalar.activation(
                        atb_t[:, half * 4:(half + 1) * 4, :], atpf,
                        mybir.ActivationFunctionType.Copy)
                # W1 apply + bias + relu -> emb2T; then xp2/a2 contraction
                o1 = o1ps.tile([128, 8, 128], F32, tag="o1")
                atb_r = atb_t.rearrange("p c (n h) -> p h c n", h=8)
                for h in range(8):
                    nc.tensor.matmul(
                        out=o1[:, h, :],
                        lhsT=w1t_sb[:, h, :],
                        rhs=atb_r[:, h, :, :],
                        start=True, stop=True)
                e2 = e2tp.tile([128, 8, 128], F16, tag="e2")
                if b1_zero:
                    nc.vector.tensor_scalar_max(e2, o1, 0.0)
                else:
                    t1 = e2tp.tile([128, 8, 128], F32, tag="t1")
                    b1_rep = bass.AP(
                        tensor=b1c_sb.tensor, offset=b1c_sb.offset,
                        ap=[b1c_sb.ap[0], b1c_sb.ap[1], [0, 128]])
                    nc.vector.tensor_tensor(out=t1, in0=o1, in1=b1_rep,
                                            op=mybir.AluOpType.add)
                    nc.vector.tensor_scalar_max(e2, t1, 0.0)
                x2 = x2ps.tile([128, 132], F32, tag="x2")
                for h in range(8):
                    nc.tensor.matmul(out=x2[:, 0:130], lhsT=e2[:, h, :],
                                     rhs=w2tv_sb[:, h, 0:130],
                                     start=(h == 0), stop=(h == 7))
                x2_sb = xsb.tile([128, 130], F32, tag="x2sb")
                nc.scalar.activation(x2_sb, x2[:, 0:130],
                                     mybir.ActivationFunctionType.Copy)
                nc.sync.dma_start(out=t_xp2o[wb * 128:(wb + 1) * 128, :],
                                  in_=x2_sb)
    nc.compile()
    return nc


# ----------------------------------------------------------------------------
# launch C: GAT layer 2 + final linear
# ----------------------------------------------------------------------------

def _build_launch_c(nchunks):
    assert nchunks % GB == 0
    nsn = nchunks * 32

    nc = bacc.Bacc("TRN2", target_bir_lowering=False, debug=False)
    t_gx = nc.dram_tensor("gx2", [128, nchunks * 132], F16,
                          kind="ExternalInput").ap()
    t_ap = nc.dram_tensor("apn2", [128, nchunks * 2], F32,
                          kind="ExternalInput").ap()
    t_mask = nc.dram_tensor("mask012", [128, nchunks * 32], F16,
                            kind="ExternalInput").ap()
    t_mx2 = nc.dram_tensor("mx2c", [128, 1], F32, kind="ExternalInput").ap()
    t_oWT = nc.dram_tensor("outWT", [128, 128], F16, kind="ExternalInput").ap()
    t_ob = nc.dram_tensor("outb", [128, 1], F32, kind="ExternalInput").ap()
    t_b2 = nc.dram_tensor("b2c", [128, 1], F32, kind="ExternalInput").ap()
    t_id = nc.dram_tensor("ident2", [128, 128], F16, kind="ExternalInput").ap()
    t_lgo = nc.dram_tensor("lgo", [128, nsn], F32, kind="ExternalOutput").ap()

    with tile.TileContext(nc) as tc:
        with (
            tc.tile_pool(name="singles", bufs=1) as singles,
            tc.tile_pool(name="gt", bufs=2) as gt,
            tc.tile_pool(name="at", bufs=2) as at,
            tc.tile_pool(name="mt", bufs=2) as mt,
            tc.tile_pool(name="et", bufs=2) as et,
            tc.tile_pool(name="pt", bufs=2) as pt,
            tc.tile_pool(name="asb", bufs=2) as asb,
            tc.tile_pool(name="rr", bufs=4) as rr,
            tc.tile_pool(name="lg", bufs=2) as lgp,
            tc.tile_pool(name="aggps", bufs=4, space="PSUM") as aggps,
            tc.tile_pool(name="atps", bufs=2, space="PSUM") as atps,
            tc.tile_pool(name="lgps", bufs=2, space="PSUM") as lgps,
        ):
            oWT_sb = singles.tile([128, 128], F16)
            nc.sync.dma_start(out=oWT_sb, in_=t_oWT)
            ob_sb = singles.tile([128, 1], F32)
            nc.sync.dma_start(out=ob_sb, in_=t_ob)
            b2_sb = singles.tile([128, 1], F32)
            nc.sync.dma_start(out=b2_sb, in_=t_b2)
            id_sb = singles.tile([128, 128], F16)
            nc.sync.dma_start(out=id_sb, in_=t_id)
            mx2_sb = singles.tile([128, 1], F32)
            nc.sync.dma_start(out=mx2_sb, in_=t_mx2)
            e3t_sb = singles.tile([128, nsn], F16)

            ngb = nchunks // GB
            for gb in range(ngb):
                gsl = slice(gb * GB * 132, (gb + 1) * GB * 132)
                g = gt.tile([128, GB, 132], F16, tag="g")
                nc.sync.dma_start(out=g, in_=t_gx[:, gsl])
                ap_ = at.tile([128, GB, 2], F32, tag="ap")
                asl = slice(gb * GB * 2, (gb + 1) * GB * 2)
                nc.sync.dma_start(out=ap_, in_=t_ap[:, asl])
                mask = mt.tile([128, GB, 32], F16, tag="mask")
                msl = slice(gb * GB * 32, (gb + 1) * GB * 32)
                nc.sync.dma_start(out=mask, in_=t_mask[:, msl])
                st = et.tile([128, GB], F32, tag="st")
                nc.vector.tensor_tensor(out=st, in0=ap_[:, :, 0],
                                        in1=ap_[:, :, 1],
                                        op=mybir.AluOpType.add)
                lk = et.tile([128, GB], F32, tag="lk")
                nc.vector.tensor_scalar_mul(lk, st, NEG)
                ee = et.tile([128, GB], F32, tag="ee")
                nc.vector.tensor_tensor(out=ee, in0=st, in1=lk,
                                        op=mybir.AluOpType.max)
                ee2 = et.tile([128, GB], F32, tag="ee2")
                nc.vector.tensor_scalar_sub(ee2, ee, mx2_sb[:, 0:1])
                ex = et.tile([128, GB], F16, tag="ex")
                nc.scalar.activation(ex, ee2, mybir.ActivationFunctionType.Exp)
                p = pt.tile([128, GB, 32], F16, tag="p")
                ex_rep = bass.AP(tensor=ex.tensor, offset=ex.offset,
                                 ap=[ex.ap[0], ex.ap[1], [0, 32]])
                nc.vector.tensor_tensor(out=p, in0=ex_rep, in1=mask,
                                        op=mybir.AluOpType.mult)

                for grp in range(GB // 4):
                    aggc = aggps.tile([128, 132], F32, tag="agg")
                    for q in range(4):
                        cb = grp * 4 + q
                        nc.tensor.matmul(out=aggc[32 * q:32 * (q + 1), 0:129],
                                         lhsT=p[:, cb, :],
                                         rhs=g[:, cb, 0:129],
                                         start=True, stop=True,
                                         tile_position=(0, 32 * q))
                    rc = rr.tile([128, 1], F32, tag="rc")
                    nc.vector.reciprocal(rc, aggc[:, 128:129])
                    a4 = asb.tile([128, 128], F16, tag="a")
                    nc.scalar.activation(a4, aggc[:, 0:128],
                                         mybir.ActivationFunctionType.Copy,
                                         scale=rc[:, 0:1])
                    atp = atps.tile([128, 128], F16, tag="atp")
                    nc.tensor.transpose(out=atp, in_=a4, identity=id_sb)
                    c0 = (gb * 4 + grp) * 128
                    nc.scalar.activation(e3t_sb[:, c0:c0 + 128], atp,
                                         mybir.ActivationFunctionType.Relu,
                                         bias=b2_sb[:, 0:1])
            # logits^T = outW.T.T @ emb3T + out_b
            nwin = nsn // 512
            for w in range(nwin):
                sl = slice(512 * w, 512 * (w + 1))
                lp = lgps.tile([128, 512], F32, tag="lg")
                nc.tensor.matmul(out=lp, lhsT=oWT_sb, rhs=e3t_sb[:, sl],
                                 start=True, stop=True)
                lsb = lgp.tile([128, 512], F32, tag="lsb")
                nc.vector.tensor_scalar_add(lsb, lp, ob_sb[:, 0:1])
                nc.sync.dma_start(out=t_lgo[:, sl], in_=lsb)
    nc.compile()
    return nc


# ----------------------------------------------------------------------------
# main entry
# ----------------------------------------------------------------------------

def kernel(**inputs):
    cs = np.ascontiguousarray(inputs["constraints_state"], np.float32)
    xs = np.ascontiguousarray(inputs["columns_state"], np.float32)
    node_W = np.asarray(inputs["node_W"], np.float32)
    node_b = np.asarray(inputs["node_b"], np.float32)
    col_W = np.asarray(inputs["col_W"], np.float32)
    col_b = np.asarray(inputs["col_b"], np.float32)
    W1 = np.asarray(inputs["W1"], np.float32)
    att_src1 = np.asarray(inputs["att_src1"], np.float32)
    att_dst1 = np.asarray(inputs["att_dst1"], np.float32)
    b1 = np.asarray(inputs["b1"], np.float32)
    W2 = np.asarray(inputs["W2"], np.float32)
    att_src2 = np.asarray(inputs["att_src2"], np.float32)
    att_dst2 = np.asarray(inputs["att_dst2"], np.float32)
    b2 = np.asarray(inputs["b2"], np.float32)
    out_W = np.asarray(inputs["out_W"], np.float32)
    out_b = np.asarray(inputs["out_b"], np.float32)
    edges = np.asarray(inputs["edges"]).astype(np.int64)

    # ---- weight folds
    W1h = W1.reshape(8, 128, 128)
    vsrc1 = np.einsum("hc,hcd->hd", att_src1, W1h).astype(np.float32)
    vdst1 = np.einsum("hc,hcd->hd", att_dst1, W1h).astype(np.float32)
    w2v = (W2.T @ np.stack([att_src2[0], att_dst2[0]], 1)).astype(np.float32)

    # ---- edges + self loops, per-core packing
    loops = np.arange(N, dtype=np.int64)
    src = np.concatenate([edges[0], loops])
    dst = np.concatenate([edges[1], loops])
    packs1, packs2 = [], []
    for core in range(N_CORES):
        lo, hi = core * SHARD, (core + 1) * SHARD
        packs1.append(_pack_edges(src, dst, lo, hi, 16))
        packs2.append(_pack_edges(src, dst, lo, hi, 32))

    def _roundup(x, m):
        return (x + m - 1) // m * m

    nc1 = _roundup(max(p["n_chunks"] for p in packs1), GB)
    nc2 = _roundup(max(p["n_chunks"] for p in packs2), GB)
    packs1 = [_pad_chunks(p, nc1) for p in packs1]
    packs2 = [_pad_chunks(p, nc2) for p in packs2]

    # ---- compile programs (cached)
    if "a" not in _programs:
        _programs["a"] = _build_launch_a()
    b1_zero = bool(np.all(b1 == 0))
    if ("b", nc1, b1_zero) not in _programs:
        _programs[("b", nc1, b1_zero)] = _build_launch_b(nc1, b1_zero)
    if ("c", nc2) not in _programs:
        _programs[("c", nc2)] = _build_launch_c(nc2)
    prog_a = _programs["a"]
    prog_b = _programs[("b", nc1, b1_zero)]
    prog_c = _programs[("c", nc2)]

    # ---- launch A
    vsV = np.concatenate([vsrc1.T, vdst1.T], 1).astype(np.float32)
    in_a = []
    for core in range(N_CORES):
        lo = core * SHARD
        if lo < N_CONS:
            feat = np.tile(cs[lo:lo + SHARD], (1, 2))
            encW = np.concatenate([node_W, np.zeros((128, 8), np.float32)], 1)
            encb_ = node_b
        else:
            feat = np.tile(xs[lo - N_CONS:lo - N_CONS + SHARD], (1, 2))
            encW = col_W
            encb_ = col_b
        encT = np.zeros((16, ENC_COLS), np.float32)
        encT[:feat.shape[1], :SHARD] = feat.T
        in_a.append({
            "encT": encT.astype(np.float16),
            "encWT": np.ascontiguousarray(encW.T, np.float16),
            "encb": encb_.reshape(128, 1).astype(np.float32),
            "vsV": vsV.astype(np.float16),
        })
    res_a = _run(prog_a, in_a, "A")
    emb1 = np.concatenate(
        [res_a.results[c]["embo"][:, :SHARD].T.astype(np.float32)
         for c in range(N_CORES)], 0)
    a1 = np.concatenate(
        [res_a.results[c]["a1o"][:, :SHARD].T for c in range(N_CORES)], 0)
    a1 = np.ascontiguousarray(a1, np.float32)               # [N, 16]

    # ---- host: expanded per-slot inputs for launch B
    emb1e = np.zeros((N, 132), np.float16)
    emb1e[:, 0:128] = emb1.astype(np.float16)
    emb1e[:, 128] = 1.0
    mx1 = _leaky_np(a1[:, 0:8].max(0) + a1[:, 8:16].max(0))

    ident = np.eye(128, dtype=np.float16)
    w1t = np.ascontiguousarray(W1h.transpose(2, 0, 1), np.float16)
    w2tv = np.zeros((128, 8, 132), np.float16)
    w2tv[:, :, 0:128] = W2.T.reshape(8, 128, 128).transpose(1, 0, 2)
    w2tv[:, :, 128:130] = w2v.reshape(8, 128, 2).transpose(1, 0, 2)
    b1c = np.ascontiguousarray(b1.reshape(8, 128).T, np.float32)

    in_b = []
    for core in range(N_CORES):
        pk = packs1[core]
        nc_ = pk["n_chunks"]
        apn = np.concatenate([
            a1[pk["src_idx"], 0:8], a1[pk["dst_idx"], 8:16]], 1)
        apn = np.ascontiguousarray(
            apn.reshape(nc_, 128, 16).transpose(1, 0, 2).reshape(128, -1),
            np.float32)
        in_b.append({
            "gx": _expand_slots(pk, emb1e, np.float16),
            "apn": apn,
            "mask01": _mask01(pk, np.float16),
            "mxr": np.tile(mx1, (128, 1)).astype(np.float32),
            "w1t": w1t, "w2tv": w2tv, "b1c": b1c, "ident": ident,
        })
    res_b = _run(prog_b, in_b, "B")

    # ---- host: assemble xp2 / a2 tables
    tab2e = np.zeros((N, 132), np.float16)
    tab2e[:, 128] = 1.0
    a2 = np.zeros((N, 2), np.float32)
    for core in range(N_CORES):
        nm = packs1[core]["node_map"]
        valid = nm >= 0
        xo = res_b.results[core]["xp2o"]
        tab2e[nm[valid], 0:128] = xo[valid, 0:128].astype(np.float16)
        a2[nm[valid]] = xo[valid, 128:130]
    mx2 = _leaky_np(np.array(
        [a2[:, 0].max() + a2[:, 1].max()], np.float32))

    in_c = []
    for core in range(N_CORES):
        pk = packs2[core]
        nc_ = pk["n_chunks"]
        apn2 = np.stack([a2[pk["src_idx"], 0], a2[pk["dst_idx"], 1]], 1)
        apn2 = np.ascontiguousarray(
            apn2.reshape(nc_, 128, 2).transpose(1, 0, 2).reshape(128, -1),
            np.float32)
        in_c.append({
            "gx2": _expand_slots(pk, tab2e, np.float16),
            "apn2": apn2,
            "mask012": _mask01(pk, np.float16),
            "mx2c": np.full((128, 1), mx2[0], np.float32),
            "outWT": np.ascontiguousarray(out_W.T, np.float16),
            "outb": out_b.reshape(128, 1).astype(np.float32),
            "b2c": b2.reshape(128, 1).astype(np.float32),
            "ident2": ident,
        })
    res_c = _run(prog_c, in_c, "C")

    logits = np.zeros((N, 128), np.float32)
    for core in range(N_CORES):
        nm = packs2[core]["node_map"]
        valid = nm >= 0
        logits[nm[valid]] = res_c.results[core]["lgo"][:, valid].T

    return logits[-N_COLS:].astype(np.float32)


_trace = {"enable": False, "dir": None, "exec_ns": {}}


def _run(prog, in_maps, tag):
    kwargs = {}
    if _trace["enable"]:
        import os
        d = os.path.join(_trace["dir"], tag)
        os.makedirs(d, exist_ok=True)
        kwargs = dict(trace=True, tmpdir=d)
    res = run_bass_kernel_spmd(prog, in_maps, core_ids=list(range(N_CORES)),
                               **kwargs)
    _trace["exec_ns"][tag] = res.exec_time_ns
    return res
